# revision 1
# baseline (speedup 1.0000x reference)
"""Trainium2 Bass kernel for nn_CNNT_enhanced_denoising_runtime_53704271069472.

Computes, distributed across 8 NeuronCores:
    q/k/v = conv3x3(x, w?, b?)          (image-sharded: B*T=128 imgs, 16/core)
    att   = causal-softmax(q @ k^T / sqrt(D)) per (batch, head)
    y     = att @ v                      (head-sharded: 16 (b,head) pairs, 2/core)
    out   = conv3x3(y, wo, bo)           (image-sharded)

Three SPMD launches with host-side resharding between them. Convs are done as
matmuls over K = (3 kx-taps x 16 ch [+ ones bias row]) x 2 ky-rows = 97(+48)
against kx-pre-shifted zero-padded image planes built on the host; the 3x3
kernel's third ky row is a second accumulating matmul with an AP row offset.
Compute dtype bf16 (fp32 PSUM accumulation).
"""
import sys
import numpy as np

sys.path.insert(0, "/opt/trn_rl_repo")

import ml_dtypes  # noqa: E402
import concourse.bacc as bacc  # noqa: E402
import concourse.tile as tile  # noqa: E402
import concourse.bass as bass  # noqa: E402
from concourse import mybir, bass_utils  # noqa: E402

BF16 = mybir.dt.bfloat16
F32 = mybir.dt.float32
NPBF16 = ml_dtypes.bfloat16

B, T, C, H, W, O = 2, 64, 16, 128, 128, 16
HP, WP = H + 2, W + 2
HW = H * W
NH, HC = 8, 2
D = HC * HW
SCALE = float(1.0 / np.sqrt(np.float32(D)))
NCORES = 8
IMGS = B * T
IPC = IMGS // NCORES  # images per core
NPL = 98  # plane rows: 48 (ky0 kx-taps) + ones + 48 (ky1) + pad

_BUILD_CACHE = {}


# ---------------- device programs ----------------

def _build_l1():
    nc = bacc.Bacc("TRN2", target_bir_lowering=False, debug=False)
    planes = nc.dram_tensor("planes", (IPC, NPL, HP * WP), BF16, kind="ExternalInput")
    lhsT0 = nc.dram_tensor("lhsT0", (97, 48), BF16, kind="ExternalInput")
    lhsT1 = nc.dram_tensor("lhsT1", (48, 48), BF16, kind="ExternalInput")
    qkv = nc.dram_tensor("qkv_out", (IPC, 128, 8192), BF16, kind="ExternalOutput")

    with tile.TileContext(nc) as tc:
        with tc.tile_pool(name="w", bufs=1) as wpool, \
             tc.tile_pool(name="pl", bufs=3) as plpool, \
             tc.tile_pool(name="st", bufs=3) as stpool, \
             tc.tile_pool(name="ps", bufs=4, space="PSUM") as pspool:
            w0 = wpool.tile([97, 48], BF16, tag="w0")
            w1 = wpool.tile([48, 48], BF16, tag="w1")
            nc.sync.dma_start(w0[:], lhsT0.ap())
            nc.sync.dma_start(w1[:], lhsT1.ap())

            def rhs_view(pt, nrows, blk, ky):
                base = (blk * 4 + ky) * WP
                return pt[0:nrows, base:base + 4 * WP].rearrange(
                    "p (h w) -> p h w", w=WP)[:, :, 0:W]

            for img in range(IPC):
                pt = plpool.tile([NPL, HP * WP], BF16)
                nc.scalar.dma_start(pt[:], planes.ap()[img])
                stage = stpool.tile([128, 8192], BF16)
                for q4 in range(8):
                    ps = pspool.tile([128, 1024], F32)
                    for sub in range(2):
                        for half in range(2):
                            blk = q4 * 4 + sub * 2 + half
                            psv = ps[half * 64:half * 64 + 48,
                                     sub * 512:sub * 512 + 512]
                            nc.tensor.matmul(psv, w0[:], rhs_view(pt, 97, blk, 0),
                                             start=True, stop=False,
                                             tile_position=(0, half * 64))
                            nc.tensor.matmul(psv, w1[:], rhs_view(pt, 48, blk, 2),
                                             start=False, stop=True,
                                             tile_position=(0, half * 64))
                    nc.vector.tensor_copy(stage[:, q4 * 1024:(q4 + 1) * 1024], ps[:])
                nc.sync.dma_start(qkv.ap()[img], stage[:])
    nc.compile()
    return nc


def _build_l2():
    nc = bacc.Bacc("TRN2", target_bir_lowering=False, debug=False)
    qks = nc.dram_tensor("qks", (8, T, HW), BF16, kind="ExternalInput")
    vs = nc.dram_tensor("vs", (4, T, HW), BF16, kind="ExternalInput")
    mask = nc.dram_tensor("mask", (T, T), F32, kind="ExternalInput")
    ident = nc.dram_tensor("ident", (T, T), BF16, kind="ExternalInput")
    ys = nc.dram_tensor("ys", (2, 128, HW), BF16, kind="ExternalOutput")

    with tile.TileContext(nc) as tc:
        with tc.tile_pool(name="cst", bufs=1) as cst, \
             tc.tile_pool(name="qk", bufs=6) as qkpool, \
             tc.tile_pool(name="sm", bufs=2) as smpool, \
             tc.tile_pool(name="v", bufs=8) as vpool, \
             tc.tile_pool(name="yst", bufs=2) as ypool, \
             tc.tile_pool(name="pst", bufs=1, space="PSUM") as pstpool, \
             tc.tile_pool(name="psy", bufs=3, space="PSUM") as psypool, \
             tc.tile_pool(name="psl", bufs=1, space="PSUM") as pslpool:
            mask_t = cst.tile([T, T], F32, tag="mask")
            nc.sync.dma_start(mask_t[:], mask.ap())
            id_t = cst.tile([T, T], BF16, tag="ident")
            nc.sync.dma_start(id_t[:], ident.ap())

            lg_ps = [pslpool.tile([128, 128], F32, tag=f"lg{h}", name=f"lg{h}")
                     for h in range(2)]
            for blk in range(128):
                qkt = qkpool.tile([128, 512], BF16, tag="qkT")
                src = qks.ap()[:, :, blk * 128:(blk + 1) * 128].rearrange("c t p -> (c t) p")
                nc.sync.dma_start_transpose(qkt[:], src)
                for h in range(2):
                    nc.tensor.matmul(lg_ps[h][:],
                                     qkt[:, h * 128:(h + 1) * 128],
                                     qkt[:, 256 + h * 128:256 + (h + 1) * 128],
                                     start=(blk == 0), stop=(blk == 127))

            attTs = []
            for h in range(2):
                lg = smpool.tile([T, T], F32, tag="lg")
                nc.vector.tensor_copy(lg[:], lg_ps[h][0:64, 0:64])
                nc.vector.tensor_add(lg[:], lg[:], lg_ps[h][64:128, 64:128])
                nc.vector.tensor_scalar(lg[:], lg[:], SCALE, None,
                                        op0=mybir.AluOpType.mult)
                nc.vector.tensor_add(lg[:], lg[:], mask_t[:])
                mx = smpool.tile([T, 1], F32, tag="mx")
                nc.vector.reduce_max(mx[:], lg[:], axis=mybir.AxisListType.X, negate=True)
                e = smpool.tile([T, T], F32, tag="e")
                sm_acc = smpool.tile([T, 1], F32, tag="smacc")
                nc.scalar.activation(e[:], lg[:], mybir.ActivationFunctionType.Exp,
                                     bias=mx[:], scale=1.0, accum_out=sm_acc[:])
                rc = smpool.tile([T, 1], F32, tag="rc")
                nc.vector.reciprocal(rc[:], sm_acc[:])
                att = smpool.tile([T, T], BF16, tag="att")
                nc.vector.tensor_scalar(att[:], e[:], rc[:], None,
                                        op0=mybir.AluOpType.mult)
                ps_t = pstpool.tile([T, T], BF16, tag="pst")
                nc.tensor.transpose(ps_t[:], att[:], id_t[:])
                attT = smpool.tile([128, T], BF16, tag=f"attT{h}", name=f"attT{h}")
                nc.vector.tensor_copy(attT[0:64, :], ps_t[:])
                nc.vector.tensor_copy(attT[64:128, :], ps_t[:])
                attTs.append(attT)

            for p in range(2):
                yst = ypool.tile([128, HW], BF16, tag="yst")
                for vb in range(4):
                    vt = vpool.tile([128, 4096], BF16, tag="vt")
                    src_v = vs.ap()[2 * p:2 * p + 2, :, vb * 4096:(vb + 1) * 4096]
                    nc.scalar.dma_start(vt[:], src_v.rearrange("c t p -> (c t) p"))
                    for ci in range(2):
                        attT = attTs[p]
                        for j in range(8):
                            ps_y = psypool.tile([T, 512], F32, tag="psy")
                            nc.tensor.matmul(ps_y[:], attT[ci * 64:ci * 64 + 64, :],
                                             vt[ci * 64:ci * 64 + 64, j * 512:(j + 1) * 512],
                                             start=True, stop=True)
                            col = vb * 4096 + j * 512
                            nc.vector.tensor_copy(
                                yst[ci * 64:ci * 64 + 64, col:col + 512], ps_y[:])
                nc.sync.dma_start(ys.ap()[p], yst[:])
    nc.compile()
    return nc


def _build_l3():
    nc = bacc.Bacc("TRN2", target_bir_lowering=False, debug=False)
    planes = nc.dram_tensor("planes", (IPC, NPL, HP * WP), BF16, kind="ExternalInput")
    lhsT0 = nc.dram_tensor("lhsT0", (97, 16), BF16, kind="ExternalInput")
    lhsT1 = nc.dram_tensor("lhsT1", (48, 16), BF16, kind="ExternalInput")
    out = nc.dram_tensor("out", (IPC, 80, 8192), F32, kind="ExternalOutput")

    with tile.TileContext(nc) as tc:
        with tc.tile_pool(name="w", bufs=1) as wpool, \
             tc.tile_pool(name="pl", bufs=3) as plpool, \
             tc.tile_pool(name="st", bufs=3) as stpool, \
             tc.tile_pool(name="ps", bufs=4, space="PSUM") as pspool:
            w0 = wpool.tile([97, 16], BF16, tag="w0")
            w1 = wpool.tile([48, 16], BF16, tag="w1")
            nc.sync.dma_start(w0[:], lhsT0.ap())
            nc.sync.dma_start(w1[:], lhsT1.ap())

            def rhs_view(pt, nrows, blk, ky):
                base = (blk * 4 + ky) * WP
                return pt[0:nrows, base:base + 4 * WP].rearrange(
                    "p (h w) -> p h w", w=WP)[:, :, 0:W]

            for img in range(IPC):
                pt = plpool.tile([NPL, HP * WP], BF16)
                nc.scalar.dma_start(pt[:], planes.ap()[img])
                stage = stpool.tile([128, 8192], F32)
                for q4 in range(8):
                    ps = pspool.tile([128, 1024], F32)
                    for sub in range(2):
                        for half in range(2):
                            blk = q4 * 4 + sub * 2 + half
                            po = half * 64
                            psv = ps[po:po + 16, sub * 512:sub * 512 + 512]
                            nc.tensor.matmul(psv, w0[:], rhs_view(pt, 97, blk, 0),
                                             start=True, stop=False,
                                             tile_position=(0, po))
                            nc.tensor.matmul(psv, w1[:], rhs_view(pt, 48, blk, 2),
                                             start=False, stop=True,
                                             tile_position=(0, po))
                    nc.vector.tensor_copy(stage[0:80, q4 * 1024:(q4 + 1) * 1024],
                                          ps[0:80, :])
                nc.sync.dma_start(out.ap()[img], stage[0:80, :])
    nc.compile()
    return nc


def _get(name):
    if name not in _BUILD_CACHE:
        _BUILD_CACHE[name] = {"l1": _build_l1, "l2": _build_l2, "l3": _build_l3}[name]()
    return _BUILD_CACHE[name]


# ---------------- host-side packing ----------------

def _build_planes(imgs_chw):
    """imgs_chw: [N, 16, H, W] float32-like -> [N, 98, HP*WP] bf16."""
    N = imgs_chw.shape[0]
    xpad = np.zeros((N, C, HP, WP), np.float32)
    xpad[:, :, 1:H + 1, 1:W + 1] = imgs_chw.astype(np.float32)
    flat = xpad.reshape(N, C, HP * WP)
    p = np.zeros((N, NPL, HP * WP), np.float32)
    p[:, 0:16] = flat
    p[:, 16:32, :-1] = flat[:, :, 1:]
    p[:, 32:48, :-2] = flat[:, :, 2:]
    p[:, 48] = 1.0
    p[:, 49:97, :-WP] = p[:, 0:48, WP:]
    return p.astype(NPBF16)


def _build_lhsT(ws, bs):
    """ws: list of [O,C,3,3]; bs: list of [O] -> lhsT0 [97, 16*len], lhsT1 [48, 16*len]."""
    n = len(ws)
    m = np.zeros((3, 49, 16 * n), np.float32)
    for j, (w, b) in enumerate(zip(ws, bs)):
        for ky in range(3):
            for kx in range(3):
                m[ky, kx * 16:(kx + 1) * 16, j * 16:(j + 1) * 16] = w[:, :, ky, kx].T
        m[1, 48, j * 16:(j + 1) * 16] = b
    l0 = np.zeros((97, 16 * n), np.float32)
    l0[0:48] = m[0][0:48]
    l0[48] = m[1][48]
    l0[49:97] = m[1][0:48]
    return l0.astype(NPBF16), m[2][0:48].astype(NPBF16)


def _unpack_qkv(qkv_out):
    """[N,128,8192] bf16 -> q,k,v each [N,16,HW].

    blk = q4*4 + sub*2 + half lives at stage rows half*64(+48), col q4*1024+sub*512."""
    N = qkv_out.shape[0]
    s = qkv_out.reshape(N, 128, 8, 2, 512)       # [N, p, q4, sub, 512]
    out = np.empty((N, 48, 8, 2, 2, 512), qkv_out.dtype)  # [N, c, q4, sub, half, 512]
    out[..., 0, :] = s[:, 0:48]
    out[..., 1, :] = s[:, 64:112]
    out = out.reshape(N, 48, HW)
    return out[:, 0:16], out[:, 16:32], out[:, 32:48]


def _unpack_l3(o):
    """[N,80,8192] f32 -> [N,16,HW].

    blk = q4*4 + sub*2 + half lives at row (half*64)+c, col q4*1024 + sub*512
    (rows 16-63 are junk from the spanning psum copy)."""
    N = o.shape[0]
    s = o.reshape(N, 80, 8, 2, 512)      # [N, row, q4, sub, 512]
    out = np.empty((N, 16, 32, 512), o.dtype)
    for q4 in range(8):
        for sub in range(2):
            for half in range(2):
                blk = q4 * 4 + sub * 2 + half
                out[:, :, blk] = s[:, half * 64:half * 64 + 16, q4, sub]
    return np.ascontiguousarray(out).reshape(N, 16, HW)


# ---------------- top level ----------------

def kernel(x, wq, bq, wk, bk, wv, bv, wo, bo):
    x, wq, bq, wk, bk, wv, bv, wo, bo = (
        np.asarray(a, np.float32) for a in (x, wq, bq, wk, bk, wv, bv, wo, bo))
    ximg = x.reshape(IMGS, C, H, W)
    cores = list(range(NCORES))

    # ---- L1: q/k/v convs, image-sharded
    l0, l1 = _build_lhsT([wq, wk, wv], [bq, bk, bv])
    in_maps = [{"planes": _build_planes(ximg[c * IPC:(c + 1) * IPC]),
                "lhsT0": l0, "lhsT1": l1} for c in cores]
    res1 = bass_utils.run_bass_kernel_spmd(_get("l1"), in_maps, core_ids=cores)

    # assemble channel-major [B, 16, T, HW] bf16
    q_all = np.empty((B, 16, T, HW), NPBF16)
    k_all = np.empty_like(q_all)
    v_all = np.empty_like(q_all)
    for c in cores:
        q, k, v = _unpack_qkv(res1.results[c]["qkv_out"])
        b0 = (c * IPC) // T
        t0 = (c * IPC) % T
        q_all[b0, :, t0:t0 + IPC] = q.transpose(1, 0, 2)
        k_all[b0, :, t0:t0 + IPC] = k.transpose(1, 0, 2)
        v_all[b0, :, t0:t0 + IPC] = v.transpose(1, 0, 2)

    # ---- L2: attention, head-sharded (2 heads = 4 channels per core)
    mask = np.triu(np.full((T, T), -30000.0, np.float32), 1)
    ident = np.eye(T, dtype=NPBF16)
    in_maps = []
    for c in cores:
        b, g = c // 4, c % 4
        sl = slice(4 * g, 4 * g + 4)
        qks = np.concatenate([q_all[b, sl], k_all[b, sl]], axis=0)
        in_maps.append({"qks": np.ascontiguousarray(qks),
                        "vs": np.ascontiguousarray(v_all[b, sl]),
                        "mask": mask, "ident": ident})
    res2 = bass_utils.run_bass_kernel_spmd(_get("l2"), in_maps, core_ids=cores)

    y_all = np.empty((B, 16, T, HW), NPBF16)
    for c in cores:
        b, g = c // 4, c % 4
        ys = res2.results[c]["ys"]
        for p in range(2):
            y_all[b, 4 * g + 2 * p] = ys[p, 0:64]
            y_all[b, 4 * g + 2 * p + 1] = ys[p, 64:128]

    # ---- L3: output conv, image-sharded
    yimg = y_all.astype(np.float32).transpose(0, 2, 1, 3).reshape(IMGS, 16, H, W)
    l0o, l1o = _build_lhsT([wo], [bo])
    in_maps = [{"planes": _build_planes(yimg[c * IPC:(c + 1) * IPC]),
                "lhsT0": l0o, "lhsT1": l1o} for c in cores]
    res3 = bass_utils.run_bass_kernel_spmd(_get("l3"), in_maps, core_ids=cores)

    out = np.concatenate([_unpack_l3(res3.results[c]["out"]) for c in cores])
    return np.ascontiguousarray(out.reshape(B, T, O, H, W))



# revision 5
# speedup vs baseline: 1.2500x; 1.2500x over previous
"""Trainium2 Bass kernel for nn_CNNT_enhanced_denoising_runtime_53704271069472.

Computes, distributed across 8 NeuronCores:
    q/k/v = conv3x3(x, w?, b?)          (image-sharded: B*T=128 imgs, 16/core)
    att   = causal-softmax(q @ k^T / sqrt(D)) per (batch, head)
    y     = att @ v                      (head-sharded: 16 (b,head) pairs, 2/core)
    out   = conv3x3(y, wo, bo)           (image-sharded)

Three SPMD launches with host-side resharding between them. Convs are done as
matmuls over K = (3 kx-taps x 16 ch [+ ones bias row]) x 2 ky-rows = 97(+48)
against kx-pre-shifted zero-padded image planes built on the host; the 3x3
kernel's third ky row is a second accumulating matmul with an AP row offset.
Compute dtype bf16 (fp32 PSUM accumulation).
"""
import sys
import numpy as np

sys.path.insert(0, "/opt/trn_rl_repo")

import ml_dtypes  # noqa: E402
import concourse.bacc as bacc  # noqa: E402
import concourse.tile as tile  # noqa: E402
import concourse.bass as bass  # noqa: E402
from concourse import mybir, bass_utils  # noqa: E402

BF16 = mybir.dt.bfloat16
F32 = mybir.dt.float32
NPBF16 = ml_dtypes.bfloat16

B, T, C, H, W, O = 2, 64, 16, 128, 128, 16
HP, WP = H + 2, W + 2
HW = H * W
NH, HC = 8, 2
D = HC * HW
SCALE = float(1.0 / np.sqrt(np.float32(D)))
NCORES = 8
IMGS = B * T
IPC = IMGS // NCORES  # images per core
NPL = 98  # plane rows: 48 (ky0 kx-taps) + ones + 48 (ky1) + pad
PL3 = HP * WP + 2  # l3 plane free size (2 slack for o=2 shift at last band)

_BUILD_CACHE = {}


# ---------------- device programs ----------------

def _build_l1():
    nc = bacc.Bacc("TRN2", target_bir_lowering=False, debug=False)
    planes = nc.dram_tensor("planes", (IPC, NPL, HP * WP), BF16, kind="ExternalInput")
    lhsT0 = nc.dram_tensor("lhsT0", (97, 48), BF16, kind="ExternalInput")
    lhsT1 = nc.dram_tensor("lhsT1", (48, 48), BF16, kind="ExternalInput")
    qkv = nc.dram_tensor("qkv_out", (IPC, 128, 8192), BF16, kind="ExternalOutput")

    with tile.TileContext(nc) as tc:
        with tc.tile_pool(name="w", bufs=1) as wpool, \
             tc.tile_pool(name="pl", bufs=3) as plpool, \
             tc.tile_pool(name="st", bufs=3) as stpool, \
             tc.tile_pool(name="ps", bufs=4, space="PSUM") as pspool:
            w0 = wpool.tile([97, 48], BF16, tag="w0")
            w1 = wpool.tile([48, 48], BF16, tag="w1")
            nc.sync.dma_start(w0[:], lhsT0.ap())
            nc.sync.dma_start(w1[:], lhsT1.ap())

            def rhs_view(pt, nrows, blk, ky):
                base = (blk * 4 + ky) * WP
                return pt[0:nrows, base:base + 4 * WP].rearrange(
                    "p (h w) -> p h w", w=WP)[:, :, 0:W]

            for img in range(IPC):
                pt = plpool.tile([NPL, HP * WP], BF16)
                nc.scalar.dma_start(pt[:], planes.ap()[img])
                stage = stpool.tile([128, 8192], BF16)
                for q4 in range(8):
                    ps = pspool.tile([128, 1024], F32)
                    for sub in range(2):
                        for half in range(2):
                            blk = q4 * 4 + sub * 2 + half
                            psv = ps[half * 64:half * 64 + 48,
                                     sub * 512:sub * 512 + 512]
                            nc.tensor.matmul(psv, w0[:], rhs_view(pt, 97, blk, 0),
                                             start=True, stop=False,
                                             tile_position=(0, half * 64))
                            nc.tensor.matmul(psv, w1[:], rhs_view(pt, 48, blk, 2),
                                             start=False, stop=True,
                                             tile_position=(0, half * 64))
                    nc.vector.tensor_copy(stage[:, q4 * 1024:(q4 + 1) * 1024], ps[:])
                nc.sync.dma_start(qkv.ap()[img], stage[:])
    nc.compile()
    return nc


def _build_l2():
    nc = bacc.Bacc("TRN2", target_bir_lowering=False, debug=False)
    qks = nc.dram_tensor("qks", (8, T, HW), BF16, kind="ExternalInput")
    vs = nc.dram_tensor("vs", (4, T, HW), BF16, kind="ExternalInput")
    mask = nc.dram_tensor("mask", (T, T), F32, kind="ExternalInput")
    ident = nc.dram_tensor("ident", (T, T), BF16, kind="ExternalInput")
    ys = nc.dram_tensor("ys", (2, 128, HW), BF16, kind="ExternalOutput")

    with tile.TileContext(nc) as tc:
        with tc.tile_pool(name="cst", bufs=1) as cst, \
             tc.tile_pool(name="qk", bufs=6) as qkpool, \
             tc.tile_pool(name="sm", bufs=2) as smpool, \
             tc.tile_pool(name="v", bufs=8) as vpool, \
             tc.tile_pool(name="yst", bufs=2) as ypool, \
             tc.tile_pool(name="pst", bufs=1, space="PSUM") as pstpool, \
             tc.tile_pool(name="psy", bufs=3, space="PSUM") as psypool, \
             tc.tile_pool(name="psl", bufs=1, space="PSUM") as pslpool:
            mask_t = cst.tile([T, T], F32, tag="mask")
            nc.sync.dma_start(mask_t[:], mask.ap())
            id_t = cst.tile([T, T], BF16, tag="ident")
            nc.sync.dma_start(id_t[:], ident.ap())

            lg_ps = [pslpool.tile([128, 128], F32, tag=f"lg{h}", name=f"lg{h}")
                     for h in range(2)]
            for blk in range(128):
                qkt = qkpool.tile([128, 512], BF16, tag="qkT")
                src = qks.ap()[:, :, blk * 128:(blk + 1) * 128].rearrange("c t p -> (c t) p")
                nc.sync.dma_start_transpose(qkt[:], src)
                for h in range(2):
                    nc.tensor.matmul(lg_ps[h][:],
                                     qkt[:, h * 128:(h + 1) * 128],
                                     qkt[:, 256 + h * 128:256 + (h + 1) * 128],
                                     start=(blk == 0), stop=(blk == 127))

            attTs = []
            for h in range(2):
                lg = smpool.tile([T, T], F32, tag="lg")
                nc.vector.tensor_copy(lg[:], lg_ps[h][0:64, 0:64])
                nc.vector.tensor_add(lg[:], lg[:], lg_ps[h][64:128, 64:128])
                nc.vector.tensor_scalar(lg[:], lg[:], SCALE, None,
                                        op0=mybir.AluOpType.mult)
                nc.vector.tensor_add(lg[:], lg[:], mask_t[:])
                mx = smpool.tile([T, 1], F32, tag="mx")
                nc.vector.reduce_max(mx[:], lg[:], axis=mybir.AxisListType.X, negate=True)
                e = smpool.tile([T, T], F32, tag="e")
                sm_acc = smpool.tile([T, 1], F32, tag="smacc")
                nc.scalar.activation(e[:], lg[:], mybir.ActivationFunctionType.Exp,
                                     bias=mx[:], scale=1.0, accum_out=sm_acc[:])
                rc = smpool.tile([T, 1], F32, tag="rc")
                nc.vector.reciprocal(rc[:], sm_acc[:])
                att = smpool.tile([T, T], BF16, tag="att")
                nc.vector.tensor_scalar(att[:], e[:], rc[:], None,
                                        op0=mybir.AluOpType.mult)
                ps_t = pstpool.tile([T, T], BF16, tag="pst")
                nc.tensor.transpose(ps_t[:], att[:], id_t[:])
                attT = smpool.tile([128, T], BF16, tag=f"attT{h}", name=f"attT{h}")
                nc.vector.tensor_copy(attT[0:64, :], ps_t[:])
                nc.vector.tensor_copy(attT[64:128, :], ps_t[:])
                attTs.append(attT)

            for p in range(2):
                yst = ypool.tile([128, HW], BF16, tag="yst")
                for vb in range(4):
                    vt = vpool.tile([128, 4096], BF16, tag="vt")
                    src_v = vs.ap()[2 * p:2 * p + 2, :, vb * 4096:(vb + 1) * 4096]
                    nc.scalar.dma_start(vt[:], src_v.rearrange("c t p -> (c t) p"))
                    for ci in range(2):
                        attT = attTs[p]
                        for j in range(8):
                            ps_y = psypool.tile([T, 512], F32, tag="psy")
                            nc.tensor.matmul(ps_y[:], attT[ci * 64:ci * 64 + 64, :],
                                             vt[ci * 64:ci * 64 + 64, j * 512:(j + 1) * 512],
                                             start=True, stop=True)
                            col = vb * 4096 + j * 512
                            nc.vector.tensor_copy(
                                yst[ci * 64:ci * 64 + 64, col:col + 512], ps_y[:])
                nc.sync.dma_start(ys.ap()[p], yst[:])
    nc.compile()
    return nc


def _build_l3():
    """o-conv, image-sharded, bf16: partitions (img4, d2, c16), M (img4, j2, och16)=128.

    6 accumulating passes per psum tile: ky in {0,1,2} x o in {0,2}; rhs is the
    plane tile at AP offset (h+ky)*WP + o with col-pair stride 2. Bias added on
    host afterwards."""
    nc = bacc.Bacc("TRN2", target_bir_lowering=False, debug=False)
    planes = nc.dram_tensor("planes", (4, 128, PL3), BF16, kind="ExternalInput")
    lhsT = nc.dram_tensor("lhsT", (128, 6 * 128), BF16, kind="ExternalInput")
    out = nc.dram_tensor("out", (4, 128, 8192), BF16, kind="ExternalOutput")

    with tile.TileContext(nc) as tc:
        with tc.tile_pool(name="w", bufs=1) as wpool, \
             tc.tile_pool(name="pl", bufs=2) as plpool, \
             tc.tile_pool(name="st", bufs=2) as stpool, \
             tc.tile_pool(name="ps", bufs=4, space="PSUM") as pspool:
            wt = wpool.tile([128, 6 * 128], BF16, tag="wt")
            nc.sync.dma_start(wt[:], lhsT.ap())


            for g in range(4):
                pt = plpool.tile([128, PL3], BF16)
                nc.scalar.dma_start(pt[:], planes.ap()[g])
                stage = stpool.tile([128, 8192], BF16)
                for band in range(16):
                    ps = pspool.tile([128, 512], F32)
                    psv = ps[:].rearrange("p (h w) -> p h w", w=64)
                    first = True
                    for ky in range(3):
                        for oi, o in enumerate((0, 2)):
                            off = (band * 8 + ky) * WP + o
                            rhs = pt[0:128, off:off + 8 * WP].rearrange(
                                "p (h w) -> p h w", w=WP)[:, :, 0:128:2]
                            pi = ky * 2 + oi
                            nc.tensor.matmul(psv, wt[:, pi * 128:(pi + 1) * 128],
                                             rhs, start=first, stop=(ky == 2 and oi == 1))
                            first = False
                    if band % 2 == 0:
                        nc.vector.tensor_copy(stage[:, band * 512:(band + 1) * 512], ps[:])
                    else:
                        nc.scalar.activation(stage[:, band * 512:(band + 1) * 512], ps[:],
                                             mybir.ActivationFunctionType.Copy)
                nc.sync.dma_start(out.ap()[g], stage[:])
    nc.compile()
    return nc


def _get(name):
    if name not in _BUILD_CACHE:
        _BUILD_CACHE[name] = {"l1": _build_l1, "l2": _build_l2, "l3": _build_l3}[name]()
    return _BUILD_CACHE[name]


# ---------------- host-side packing ----------------

def _build_planes(imgs_chw):
    """imgs_chw: [N, 16, H, W] float32-like -> [N, 98, HP*WP] bf16."""
    N = imgs_chw.shape[0]
    xpad = np.zeros((N, C, HP, WP), np.float32)
    xpad[:, :, 1:H + 1, 1:W + 1] = imgs_chw.astype(np.float32)
    flat = xpad.reshape(N, C, HP * WP)
    p = np.zeros((N, NPL, HP * WP), np.float32)
    p[:, 0:16] = flat
    p[:, 16:32, :-1] = flat[:, :, 1:]
    p[:, 32:48, :-2] = flat[:, :, 2:]
    p[:, 48] = 1.0
    p[:, 49:97, :-WP] = p[:, 0:48, WP:]
    return p.astype(NPBF16)


def _build_lhsT(ws, bs):
    """ws: list of [O,C,3,3]; bs: list of [O] -> lhsT0 [97, 16*len], lhsT1 [48, 16*len]."""
    n = len(ws)
    m = np.zeros((3, 49, 16 * n), np.float32)
    for j, (w, b) in enumerate(zip(ws, bs)):
        for ky in range(3):
            for kx in range(3):
                m[ky, kx * 16:(kx + 1) * 16, j * 16:(j + 1) * 16] = w[:, :, ky, kx].T
        m[1, 48, j * 16:(j + 1) * 16] = b
    l0 = np.zeros((97, 16 * n), np.float32)
    l0[0:48] = m[0][0:48]
    l0[48] = m[1][48]
    l0[49:97] = m[1][0:48]
    return l0.astype(NPBF16), m[2][0:48].astype(NPBF16)


def _unpack_qkv(qkv_out):
    """[N,128,8192] bf16 -> q,k,v each [N,16,HW].

    blk = q4*4 + sub*2 + half lives at stage rows half*64(+48), col q4*1024+sub*512."""
    N = qkv_out.shape[0]
    s = qkv_out.reshape(N, 128, 8, 2, 512)       # [N, p, q4, sub, 512]
    out = np.empty((N, 48, 8, 2, 2, 512), qkv_out.dtype)  # [N, c, q4, sub, half, 512]
    out[..., 0, :] = s[:, 0:48]
    out[..., 1, :] = s[:, 64:112]
    out = out.reshape(N, 48, HW)
    return out[:, 0:16], out[:, 16:32], out[:, 32:48]


def _pack_l3_planes(yimg16):
    """yimg16: [16, 16, H, W] float32 -> [4, 128, HP*WP] bf16 (img4, d2, c16)."""
    ypad = np.zeros((16, C, HP, WP), np.float32)
    ypad[:, :, 1:H + 1, 1:W + 1] = yimg16
    flat = ypad.reshape(16, C, HP * WP).astype(NPBF16)
    p = np.zeros((4, 4, 2, C, PL3), NPBF16)
    p[:, :, 0, :, :HP * WP] = flat.reshape(4, 4, C, HP * WP)
    p[:, :, 1, :, :HP * WP - 1] = flat.reshape(4, 4, C, HP * WP)[..., 1:]
    return p.reshape(4, 128, PL3)


def _build_l3_lhsT(wo):
    """wo: [O, C, 3, 3] -> [6, 128, 128] bf16; row (i,d,c), col (i,j,och)."""
    m = np.zeros((3, 2, 2, C, 2, O), np.float32)  # [ky, o_i, d, c, j, och]
    for ky in range(3):
        for oi, o in enumerate((0, 2)):
            for d in range(2):
                for j in range(2):
                    kx = o + d - j
                    if 0 <= kx <= 2:
                        m[ky, oi, d, :, j, :] = wo[:, :, ky, kx].T
    l = np.zeros((3, 2, 4, 2, C, 4, 2, O), np.float32)  # [ky,oi, i,d,c, i',j,och]
    for i in range(4):
        l[:, :, i, :, :, i] = m
    return np.ascontiguousarray(
        l.reshape(6, 128, 128).transpose(1, 0, 2)).reshape(128, 6 * 128).astype(NPBF16)


def _unpack_l3(o):
    """[4, 128, 8192] bf16 -> [16, 16, H, W] float32."""
    s = o.reshape(4, 4, 2, 16, 16, 8, 64).astype(np.float32)  # g i j och band r n
    s = s.transpose(0, 1, 3, 4, 5, 6, 2)  # g i och band r n j
    return np.ascontiguousarray(s).reshape(16, 16, 128, 128)


# ---------------- top level ----------------

def kernel(x, wq, bq, wk, bk, wv, bv, wo, bo):
    x, wq, bq, wk, bk, wv, bv, wo, bo = (
        np.asarray(a, np.float32) for a in (x, wq, bq, wk, bk, wv, bv, wo, bo))
    ximg = x.reshape(IMGS, C, H, W)
    cores = list(range(NCORES))

    # ---- L1: q/k/v convs, image-sharded
    l0, l1 = _build_lhsT([wq, wk, wv], [bq, bk, bv])
    in_maps = [{"planes": _build_planes(ximg[c * IPC:(c + 1) * IPC]),
                "lhsT0": l0, "lhsT1": l1} for c in cores]
    res1 = bass_utils.run_bass_kernel_spmd(_get("l1"), in_maps, core_ids=cores)

    # assemble channel-major [B, 16, T, HW] bf16
    q_all = np.empty((B, 16, T, HW), NPBF16)
    k_all = np.empty_like(q_all)
    v_all = np.empty_like(q_all)
    for c in cores:
        q, k, v = _unpack_qkv(res1.results[c]["qkv_out"])
        b0 = (c * IPC) // T
        t0 = (c * IPC) % T
        q_all[b0, :, t0:t0 + IPC] = q.transpose(1, 0, 2)
        k_all[b0, :, t0:t0 + IPC] = k.transpose(1, 0, 2)
        v_all[b0, :, t0:t0 + IPC] = v.transpose(1, 0, 2)

    # ---- L2: attention, head-sharded (2 heads = 4 channels per core)
    mask = np.triu(np.full((T, T), -30000.0, np.float32), 1)
    ident = np.eye(T, dtype=NPBF16)
    in_maps = []
    for c in cores:
        b, g = c // 4, c % 4
        sl = slice(4 * g, 4 * g + 4)
        qks = np.concatenate([q_all[b, sl], k_all[b, sl]], axis=0)
        in_maps.append({"qks": np.ascontiguousarray(qks),
                        "vs": np.ascontiguousarray(v_all[b, sl]),
                        "mask": mask, "ident": ident})
    res2 = bass_utils.run_bass_kernel_spmd(_get("l2"), in_maps, core_ids=cores)

    y_all = np.empty((B, 16, T, HW), NPBF16)
    for c in cores:
        b, g = c // 4, c % 4
        ys = res2.results[c]["ys"]
        for p in range(2):
            y_all[b, 4 * g + 2 * p] = ys[p, 0:64]
            y_all[b, 4 * g + 2 * p + 1] = ys[p, 64:128]

    # ---- L3: output conv, image-sharded
    yimg = y_all.astype(np.float32).transpose(0, 2, 1, 3).reshape(IMGS, 16, H, W)
    l3w = _build_l3_lhsT(wo)
    in_maps = [{"planes": _pack_l3_planes(yimg[c * IPC:(c + 1) * IPC]),
                "lhsT": l3w} for c in cores]
    res3 = bass_utils.run_bass_kernel_spmd(_get("l3"), in_maps, core_ids=cores)

    out = np.concatenate([_unpack_l3(res3.results[c]["out"]) for c in cores])
    out = out + bo.reshape(1, 16, 1, 1)
    return np.ascontiguousarray(out.reshape(B, T, O, H, W))


# revision 6
# speedup vs baseline: 1.7192x; 1.3753x over previous
"""Trainium2 Bass kernel for nn_CNNT_enhanced_denoising_runtime_53704271069472.

Computes, distributed across 8 NeuronCores:
    q/k/v = conv3x3(x, w?, b?)          (image-sharded: B*T=128 imgs, 16/core)
    att   = causal-softmax(q @ k^T / sqrt(D)) per (batch, head)
    y     = att @ v                      (head-sharded: 16 (b,head) pairs, 2/core)
    out   = conv3x3(y, wo, bo)           (image-sharded)

Three SPMD launches with host-side resharding between them. Convs are done as
matmuls over K = (3 kx-taps x 16 ch [+ ones bias row]) x 2 ky-rows = 97(+48)
against kx-pre-shifted zero-padded image planes built on the host; the 3x3
kernel's third ky row is a second accumulating matmul with an AP row offset.
Compute dtype bf16 (fp32 PSUM accumulation).
"""
import sys
import numpy as np

sys.path.insert(0, "/opt/trn_rl_repo")

import ml_dtypes  # noqa: E402
import concourse.bacc as bacc  # noqa: E402
import concourse.tile as tile  # noqa: E402
import concourse.bass as bass  # noqa: E402
from concourse import mybir, bass_utils  # noqa: E402

BF16 = mybir.dt.bfloat16
F32 = mybir.dt.float32
FP8 = mybir.dt.float8e4
NPBF16 = ml_dtypes.bfloat16
NPFP8 = ml_dtypes.float8_e4m3

B, T, C, H, W, O = 2, 64, 16, 128, 128, 16
HP, WP = H + 2, W + 2
HW = H * W
NH, HC = 8, 2
D = HC * HW
SCALE = float(1.0 / np.sqrt(np.float32(D)))
NCORES = 8
IMGS = B * T
IPC = IMGS // NCORES  # images per core
NPL = 98  # plane rows: 48 (ky0 kx-taps) + ones + 48 (ky1) + pad
PL3 = HP * WP + 2  # l3 plane free size (2 slack for o=2 shift at last band)

_BUILD_CACHE = {}


# ---------------- device programs ----------------

def _build_l1():
    nc = bacc.Bacc("TRN2", target_bir_lowering=False, debug=False)
    planes = nc.dram_tensor("planes", (IPC, NPL, HP * WP), BF16, kind="ExternalInput")
    lhsT0 = nc.dram_tensor("lhsT0", (97, 48), BF16, kind="ExternalInput")
    lhsT1 = nc.dram_tensor("lhsT1", (48, 48), BF16, kind="ExternalInput")
    qkv = nc.dram_tensor("qkv_out", (IPC, 128, 8192), BF16, kind="ExternalOutput")

    with tile.TileContext(nc) as tc:
        with tc.tile_pool(name="w", bufs=1) as wpool, \
             tc.tile_pool(name="pl", bufs=3) as plpool, \
             tc.tile_pool(name="st", bufs=3) as stpool, \
             tc.tile_pool(name="ps", bufs=4, space="PSUM") as pspool:
            w0 = wpool.tile([97, 48], BF16, tag="w0")
            w1 = wpool.tile([48, 48], BF16, tag="w1")
            nc.sync.dma_start(w0[:], lhsT0.ap())
            nc.sync.dma_start(w1[:], lhsT1.ap())

            def rhs_view(pt, nrows, blk, ky):
                base = (blk * 4 + ky) * WP
                return pt[0:nrows, base:base + 4 * WP].rearrange(
                    "p (h w) -> p h w", w=WP)[:, :, 0:W]

            for img in range(IPC):
                pt = plpool.tile([NPL, HP * WP], BF16)
                nc.scalar.dma_start(pt[:], planes.ap()[img])
                stage = stpool.tile([128, 8192], BF16)
                for q4 in range(8):
                    ps = pspool.tile([128, 1024], F32)
                    for sub in range(2):
                        for half in range(2):
                            blk = q4 * 4 + sub * 2 + half
                            psv = ps[half * 64:half * 64 + 48,
                                     sub * 512:sub * 512 + 512]
                            nc.tensor.matmul(psv, w0[:], rhs_view(pt, 97, blk, 0),
                                             start=True, stop=False,
                                             tile_position=(0, half * 64))
                            nc.tensor.matmul(psv, w1[:], rhs_view(pt, 48, blk, 2),
                                             start=False, stop=True,
                                             tile_position=(0, half * 64))
                    nc.vector.tensor_copy(stage[:, q4 * 1024:(q4 + 1) * 1024], ps[:])
                nc.sync.dma_start(qkv.ap()[img], stage[:])
    nc.compile()
    return nc


def _build_l2():
    """Attention, head-sharded (2 heads/core).

    Logits: 256 accumulating K=128 matmuls per head over host-packed d-major
    fp8 qT/kT tiles. Softmax on device. att@v as 64 N=512 matmuls with a
    block-diagonal [128,128] lhsT covering both heads at once."""
    nc = bacc.Bacc("TRN2", target_bir_lowering=False, debug=False)
    qt = nc.dram_tensor("qt", (2, 128, 16384), FP8, kind="ExternalInput")
    kt = nc.dram_tensor("kt", (2, 128, 16384), FP8, kind="ExternalInput")
    vst = nc.dram_tensor("vst", (128, HW * 2), BF16, kind="ExternalInput")
    mask = nc.dram_tensor("mask", (T, T), F32, kind="ExternalInput")
    ident = nc.dram_tensor("ident", (T, T), BF16, kind="ExternalInput")
    ys = nc.dram_tensor("ys", (128, HW * 2), BF16, kind="ExternalOutput")

    with tile.TileContext(nc) as tc:
        with tc.tile_pool(name="cst", bufs=1) as cst, \
             tc.tile_pool(name="qk", bufs=1) as qkpool, \
             tc.tile_pool(name="sm", bufs=2) as smpool, \
             tc.tile_pool(name="v", bufs=3) as vpool, \
             tc.tile_pool(name="yst", bufs=3) as ypool, \
             tc.tile_pool(name="pst", bufs=2, space="PSUM") as pstpool, \
             tc.tile_pool(name="psy", bufs=4, space="PSUM") as psypool, \
             tc.tile_pool(name="psl", bufs=1, space="PSUM") as pslpool:
            mask_t = cst.tile([T, T], F32, tag="mask")
            nc.sync.dma_start(mask_t[:], mask.ap())
            id_t = cst.tile([T, T], BF16, tag="ident")
            nc.sync.dma_start(id_t[:], ident.ap())

            qtl = qkpool.tile([128, 2 * 16384], FP8, tag="qtl")
            ktl = qkpool.tile([128, 2 * 16384], FP8, tag="ktl")
            for h in range(2):
                nc.scalar.dma_start(qtl[:, h * 16384:(h + 1) * 16384], qt.ap()[h])
                nc.scalar.dma_start(ktl[:, h * 16384:(h + 1) * 16384], kt.ap()[h])

            ld = qkpool.tile([128, 128], BF16, tag="ld")
            nc.vector.memset(ld[:], 0)

            for h in range(2):
                lg_ps = pslpool.tile([T, T], F32, name=f"lg{h}")
                for ck in range(256):
                    o = h * 16384 + ck * 64
                    nc.tensor.matmul(lg_ps[:], qtl[:, o:o + 64], ktl[:, o:o + 64],
                                     start=(ck == 0), stop=(ck == 255))
                lg = smpool.tile([T, T], F32, tag="lg")
                nc.vector.tensor_scalar(lg[:], lg_ps[:], SCALE, None,
                                        op0=mybir.AluOpType.mult)
                nc.vector.tensor_add(lg[:], lg[:], mask_t[:])
                mx = smpool.tile([T, 1], F32, tag="mx")
                nc.vector.reduce_max(mx[:], lg[:], axis=mybir.AxisListType.X, negate=True)
                e = smpool.tile([T, T], F32, tag="e")
                sm_acc = smpool.tile([T, 1], F32, tag="smacc")
                nc.scalar.activation(e[:], lg[:], mybir.ActivationFunctionType.Exp,
                                     bias=mx[:], scale=1.0, accum_out=sm_acc[:])
                rc = smpool.tile([T, 1], F32, tag="rc")
                nc.vector.reciprocal(rc[:], sm_acc[:])
                att = smpool.tile([T, T], BF16, tag="att")
                nc.vector.tensor_scalar(att[:], e[:], rc[:], None,
                                        op0=mybir.AluOpType.mult)
                ps_t = pstpool.tile([T, T], BF16, tag="pst")
                nc.tensor.transpose(ps_t[:], att[:], id_t[:])
                nc.vector.tensor_copy(ld[h * 64:h * 64 + 64, h * 64:h * 64 + 64], ps_t[:])

            for blk in range(4):
                vt = vpool.tile([128, 8192], BF16, tag="vt")
                nc.scalar.dma_start(vt[:], vst.ap()[:, blk * 8192:(blk + 1) * 8192])
                yst = ypool.tile([128, 8192], BF16, tag="yst")
                for j in range(16):
                    ps_y = psypool.tile([128, 512], F32, tag="psy")
                    nc.tensor.matmul(ps_y[:], ld[:], vt[:, j * 512:(j + 1) * 512],
                                     start=True, stop=True)
                    if j % 2 == 0:
                        nc.vector.tensor_copy(yst[:, j * 512:(j + 1) * 512], ps_y[:])
                    else:
                        nc.scalar.activation(yst[:, j * 512:(j + 1) * 512], ps_y[:],
                                             mybir.ActivationFunctionType.Copy)
                nc.sync.dma_start(ys.ap()[:, blk * 8192:(blk + 1) * 8192], yst[:])
    nc.compile()
    return nc


def _build_l3():
    """o-conv, image-sharded, bf16: partitions (img4, d2, c16), M (img4, j2, och16)=128.

    6 accumulating passes per psum tile: ky in {0,1,2} x o in {0,2}; rhs is the
    plane tile at AP offset (h+ky)*WP + o with col-pair stride 2. Bias added on
    host afterwards."""
    nc = bacc.Bacc("TRN2", target_bir_lowering=False, debug=False)
    planes = nc.dram_tensor("planes", (4, 128, PL3), BF16, kind="ExternalInput")
    lhsT = nc.dram_tensor("lhsT", (128, 6 * 128), BF16, kind="ExternalInput")
    out = nc.dram_tensor("out", (4, 128, 8192), BF16, kind="ExternalOutput")

    with tile.TileContext(nc) as tc:
        with tc.tile_pool(name="w", bufs=1) as wpool, \
             tc.tile_pool(name="pl", bufs=2) as plpool, \
             tc.tile_pool(name="st", bufs=2) as stpool, \
             tc.tile_pool(name="ps", bufs=4, space="PSUM") as pspool:
            wt = wpool.tile([128, 6 * 128], BF16, tag="wt")
            nc.sync.dma_start(wt[:], lhsT.ap())


            for g in range(4):
                pt = plpool.tile([128, PL3], BF16)
                nc.scalar.dma_start(pt[:], planes.ap()[g])
                stage = stpool.tile([128, 8192], BF16)
                for band in range(16):
                    ps = pspool.tile([128, 512], F32)
                    psv = ps[:].rearrange("p (h w) -> p h w", w=64)
                    first = True
                    for ky in range(3):
                        for oi, o in enumerate((0, 2)):
                            off = (band * 8 + ky) * WP + o
                            rhs = pt[0:128, off:off + 8 * WP].rearrange(
                                "p (h w) -> p h w", w=WP)[:, :, 0:128:2]
                            pi = ky * 2 + oi
                            nc.tensor.matmul(psv, wt[:, pi * 128:(pi + 1) * 128],
                                             rhs, start=first, stop=(ky == 2 and oi == 1))
                            first = False
                    if band % 2 == 0:
                        nc.vector.tensor_copy(stage[:, band * 512:(band + 1) * 512], ps[:])
                    else:
                        nc.scalar.activation(stage[:, band * 512:(band + 1) * 512], ps[:],
                                             mybir.ActivationFunctionType.Copy)
                nc.sync.dma_start(out.ap()[g], stage[:])
    nc.compile()
    return nc


def _get(name):
    if name not in _BUILD_CACHE:
        _BUILD_CACHE[name] = {"l1": _build_l1, "l2": _build_l2, "l3": _build_l3}[name]()
    return _BUILD_CACHE[name]


# ---------------- host-side packing ----------------

def _build_planes(imgs_chw):
    """imgs_chw: [N, 16, H, W] float32-like -> [N, 98, HP*WP] bf16."""
    N = imgs_chw.shape[0]
    xpad = np.zeros((N, C, HP, WP), np.float32)
    xpad[:, :, 1:H + 1, 1:W + 1] = imgs_chw.astype(np.float32)
    flat = xpad.reshape(N, C, HP * WP)
    p = np.zeros((N, NPL, HP * WP), np.float32)
    p[:, 0:16] = flat
    p[:, 16:32, :-1] = flat[:, :, 1:]
    p[:, 32:48, :-2] = flat[:, :, 2:]
    p[:, 48] = 1.0
    p[:, 49:97, :-WP] = p[:, 0:48, WP:]
    return p.astype(NPBF16)


def _build_lhsT(ws, bs):
    """ws: list of [O,C,3,3]; bs: list of [O] -> lhsT0 [97, 16*len], lhsT1 [48, 16*len]."""
    n = len(ws)
    m = np.zeros((3, 49, 16 * n), np.float32)
    for j, (w, b) in enumerate(zip(ws, bs)):
        for ky in range(3):
            for kx in range(3):
                m[ky, kx * 16:(kx + 1) * 16, j * 16:(j + 1) * 16] = w[:, :, ky, kx].T
        m[1, 48, j * 16:(j + 1) * 16] = b
    l0 = np.zeros((97, 16 * n), np.float32)
    l0[0:48] = m[0][0:48]
    l0[48] = m[1][48]
    l0[49:97] = m[1][0:48]
    return l0.astype(NPBF16), m[2][0:48].astype(NPBF16)


def _unpack_qkv(qkv_out):
    """[N,128,8192] bf16 -> q,k,v each [N,16,HW].

    blk = q4*4 + sub*2 + half lives at stage rows half*64(+48), col q4*1024+sub*512."""
    N = qkv_out.shape[0]
    s = qkv_out.reshape(N, 128, 8, 2, 512)       # [N, p, q4, sub, 512]
    out = np.empty((N, 48, 8, 2, 2, 512), qkv_out.dtype)  # [N, c, q4, sub, half, 512]
    out[..., 0, :] = s[:, 0:48]
    out[..., 1, :] = s[:, 64:112]
    out = out.reshape(N, 48, HW)
    return out[:, 0:16], out[:, 16:32], out[:, 32:48]


def _pack_l3_planes(yimg16):
    """yimg16: [16, 16, H, W] float32 -> [4, 128, HP*WP] bf16 (img4, d2, c16)."""
    ypad = np.zeros((16, C, HP, WP), np.float32)
    ypad[:, :, 1:H + 1, 1:W + 1] = yimg16
    flat = ypad.reshape(16, C, HP * WP).astype(NPBF16)
    p = np.zeros((4, 4, 2, C, PL3), NPBF16)
    p[:, :, 0, :, :HP * WP] = flat.reshape(4, 4, C, HP * WP)
    p[:, :, 1, :, :HP * WP - 1] = flat.reshape(4, 4, C, HP * WP)[..., 1:]
    return p.reshape(4, 128, PL3)


def _build_l3_lhsT(wo):
    """wo: [O, C, 3, 3] -> [6, 128, 128] bf16; row (i,d,c), col (i,j,och)."""
    m = np.zeros((3, 2, 2, C, 2, O), np.float32)  # [ky, o_i, d, c, j, och]
    for ky in range(3):
        for oi, o in enumerate((0, 2)):
            for d in range(2):
                for j in range(2):
                    kx = o + d - j
                    if 0 <= kx <= 2:
                        m[ky, oi, d, :, j, :] = wo[:, :, ky, kx].T
    l = np.zeros((3, 2, 4, 2, C, 4, 2, O), np.float32)  # [ky,oi, i,d,c, i',j,och]
    for i in range(4):
        l[:, :, i, :, :, i] = m
    return np.ascontiguousarray(
        l.reshape(6, 128, 128).transpose(1, 0, 2)).reshape(128, 6 * 128).astype(NPBF16)


def _unpack_l3(o):
    """[4, 128, 8192] bf16 -> [16, 16, H, W] float32."""
    s = o.reshape(4, 4, 2, 16, 16, 8, 64).astype(np.float32)  # g i j och band r n
    s = s.transpose(0, 1, 3, 4, 5, 6, 2)  # g i och band r n j
    return np.ascontiguousarray(s).reshape(16, 16, 128, 128)


# ---------------- top level ----------------

def kernel(x, wq, bq, wk, bk, wv, bv, wo, bo):
    x, wq, bq, wk, bk, wv, bv, wo, bo = (
        np.asarray(a, np.float32) for a in (x, wq, bq, wk, bk, wv, bv, wo, bo))
    ximg = x.reshape(IMGS, C, H, W)
    cores = list(range(NCORES))

    # ---- L1: q/k/v convs, image-sharded
    l0, l1 = _build_lhsT([wq, wk, wv], [bq, bk, bv])
    in_maps = [{"planes": _build_planes(ximg[c * IPC:(c + 1) * IPC]),
                "lhsT0": l0, "lhsT1": l1} for c in cores]
    res1 = bass_utils.run_bass_kernel_spmd(_get("l1"), in_maps, core_ids=cores)

    # assemble channel-major [B, 16, T, HW] bf16
    q_all = np.empty((B, 16, T, HW), NPBF16)
    k_all = np.empty_like(q_all)
    v_all = np.empty_like(q_all)
    for c in cores:
        q, k, v = _unpack_qkv(res1.results[c]["qkv_out"])
        b0 = (c * IPC) // T
        t0 = (c * IPC) % T
        q_all[b0, :, t0:t0 + IPC] = q.transpose(1, 0, 2)
        k_all[b0, :, t0:t0 + IPC] = k.transpose(1, 0, 2)
        v_all[b0, :, t0:t0 + IPC] = v.transpose(1, 0, 2)

    # ---- L2: attention, head-sharded (2 heads = 4 channels per core)
    mask = np.triu(np.full((T, T), -30000.0, np.float32), 1)
    ident = np.eye(T, dtype=NPBF16)
    in_maps = []
    for c in cores:
        b, g = c // 4, c % 4
        q4 = q_all[b, 4 * g:4 * g + 4].astype(np.float32)   # [4, T, HW]
        k4 = k_all[b, 4 * g:4 * g + 4].astype(np.float32)
        v4 = v_all[b, 4 * g:4 * g + 4].astype(np.float32)

        def dmaj(a):
            # [4,T,HW] -> [2 heads, d=2*HW, T] -> packed [2, 128, 16384] fp8
            aT = a.reshape(2, 2, T, HW).transpose(0, 1, 3, 2).reshape(2, 2 * HW, T)
            return np.ascontiguousarray(
                aT.reshape(2, 256, 128, T).transpose(0, 2, 1, 3).reshape(2, 128, 256 * T)
            ).astype(NPFP8)

        vstk = np.ascontiguousarray(
            v4.reshape(2, 2, T, HW).transpose(0, 2, 1, 3).reshape(128, 2 * HW)
        ).astype(NPBF16)
        in_maps.append({"qt": dmaj(q4), "kt": dmaj(k4), "vst": vstk,
                        "mask": mask, "ident": ident})
    res2 = bass_utils.run_bass_kernel_spmd(_get("l2"), in_maps, core_ids=cores)

    y_all = np.empty((B, 16, T, HW), NPBF16)
    for c in cores:
        b, g = c // 4, c % 4
        ys = res2.results[c]["ys"].reshape(2, T, 2, HW).transpose(0, 2, 1, 3)
        y_all[b, 4 * g:4 * g + 4] = ys.reshape(4, T, HW)
        # ---- L3: output conv, image-sharded
    yimg = y_all.astype(np.float32).transpose(0, 2, 1, 3).reshape(IMGS, 16, H, W)
    l3w = _build_l3_lhsT(wo)
    in_maps = [{"planes": _pack_l3_planes(yimg[c * IPC:(c + 1) * IPC]),
                "lhsT": l3w} for c in cores]
    res3 = bass_utils.run_bass_kernel_spmd(_get("l3"), in_maps, core_ids=cores)

    out = np.concatenate([_unpack_l3(res3.results[c]["out"]) for c in cores])
    out = out + bo.reshape(1, 16, 1, 1)
    return np.ascontiguousarray(out.reshape(B, T, O, H, W))


# revision 10
# speedup vs baseline: 2.1454x; 1.2479x over previous
"""Trainium2 Bass kernel for nn_CNNT_enhanced_denoising_runtime_53704271069472.

Computes, distributed across 8 NeuronCores:
    q/k/v = conv3x3(x, w?, b?)          (image-sharded: B*T=128 imgs, 16/core)
    att   = causal-softmax(q @ k^T / sqrt(D)) per (batch, head)
    y     = att @ v                      (head-sharded: 16 (b,head) pairs, 2/core)
    out   = conv3x3(y, wo, bo)           (image-sharded)

Three SPMD launches with host-side resharding between them. Convs are done as
matmuls over K = (3 kx-taps x 16 ch [+ ones bias row]) x 2 ky-rows = 97(+48)
against kx-pre-shifted zero-padded image planes built on the host; the 3x3
kernel's third ky row is a second accumulating matmul with an AP row offset.
Compute dtype bf16 (fp32 PSUM accumulation).
"""
import sys
import numpy as np

sys.path.insert(0, "/opt/trn_rl_repo")

import ml_dtypes  # noqa: E402
import concourse.bacc as bacc  # noqa: E402
import concourse.tile as tile  # noqa: E402
import concourse.bass as bass  # noqa: E402
from concourse import mybir, bass_utils  # noqa: E402

BF16 = mybir.dt.bfloat16
F32 = mybir.dt.float32
FP8 = mybir.dt.float8e4
NPBF16 = ml_dtypes.bfloat16
NPFP8 = ml_dtypes.float8_e4m3

B, T, C, H, W, O = 2, 64, 16, 128, 128, 16
HP, WP = H + 2, W + 2
HW = H * W
NH, HC = 8, 2
D = HC * HW
SCALE = float(1.0 / np.sqrt(np.float32(D)))
NCORES = 8
IMGS = B * T
IPC = IMGS // NCORES  # images per core
NPL = 98  # plane rows: 48 (ky0 kx-taps) + ones + 48 (ky1) + pad
PL3 = HP * WP + 2  # l3 plane free size (2 slack for o=2 shift at last band)
PL1 = (HP + 1) * WP + 2  # l1 plane: extra zero row + slack for ky-pair overrun
WSC = 64.0  # fp8 weight scale 2**6, folded out at psum copy

_BUILD_CACHE = {}


# ---------------- device programs ----------------

def _build_l1():
    """qkv convs, image-sharded, fp8 DoubleRow.

    Per 2 images one tile [(xd2, i2, d2, c16)=128, PL1] fp8: xd = (x_hi|x_lo)
    dual-fp8 halves, d in {0,1} column-shifted copies, per-partition planes of
    the zero-padded image. q+k fused: DR pair=(ky,ky+1) via free-dim stride WP,
    4 passes (kygrp2 x o in {0,2}), M=128=(i2,j2,qk2,och16). v: DR pair =
    (w_hi,w_lo) via stride-0 rhs pair (weight-dual), 6 passes (ky3 x o2),
    M=64=(i2,j2,och16). Weights are pre-scaled by WSC; copies scale back.
    Biases are added on the host."""
    nc = bacc.Bacc("TRN2", target_bir_lowering=False, debug=False)
    planes = nc.dram_tensor("planes", (IPC // 2, 128, PL1), FP8, kind="ExternalInput")
    qkw = nc.dram_tensor("qkw", (128, 4 * 256), FP8, kind="ExternalInput")
    vw = nc.dram_tensor("vw", (128, 6 * 128), FP8, kind="ExternalInput")
    qk = nc.dram_tensor("qk_out", (IPC // 2, 128, 8192), FP8, kind="ExternalOutput")
    vo = nc.dram_tensor("v_out", (IPC // 2, 64, 8192), BF16, kind="ExternalOutput")

    with tile.TileContext(nc) as tc:
        with tc.tile_pool(name="w", bufs=1) as wpool, \
             tc.tile_pool(name="pl", bufs=2) as plpool, \
             tc.tile_pool(name="stq", bufs=2) as stqpool, \
             tc.tile_pool(name="stv", bufs=2) as stvpool, \
             tc.tile_pool(name="ps", bufs=3, space="PSUM") as pspool:
            qw = wpool.tile([128, 4 * 256], FP8, tag="qw")
            nc.sync.dma_start(qw[:], qkw.ap())
            vwt = wpool.tile([128, 6 * 128], FP8, tag="vw")
            nc.sync.dma_start(vwt[:], vw.ap())

            def pair_rhs(pt, off, stride):
                base = pt[0:128, off:off + 9 * WP].rearrange(
                    "p (h w) -> p h w", w=WP)[:, 0:8, 0:128:2]
                r = base.unsqueeze(1).broadcast_to((128, 2, 8, 64))
                if stride:
                    r.ap[1] = [stride, 2]
                return r

            for pr in range(IPC // 2):
                pt = plpool.tile([128, PL1], FP8)
                nc.scalar.dma_start(pt[:], planes.ap()[pr])
                stq = stqpool.tile([128, 8192], FP8, tag="stq")
                stv = stvpool.tile([64, 8192], BF16, tag="stv")
                for band in range(16):
                    psq = pspool.tile([128, 512], F32, tag="psq")
                    psqv = psq[:].rearrange("p (h w) -> p h w", w=64)
                    pi = 0
                    for grp in range(2):
                        for o in (0, 2):
                            off = (band * 8 + grp * 2) * WP + o
                            nc.tensor.matmul(
                                psqv,
                                qw[:, pi * 256:(pi + 1) * 256].rearrange(
                                    "p (two m) -> p two m", two=2),
                                pair_rhs(pt, off, WP),
                                start=(pi == 0), stop=(pi == 3),
                                perf_mode=mybir.MatmulPerfMode.DoubleRow)
                            pi += 1
                    psv = pspool.tile([64, 512], F32, tag="psv")
                    psvv = psv[:].rearrange("p (h w) -> p h w", w=64)
                    pi = 0
                    for ky in range(3):
                        for o in (0, 2):
                            off = (band * 8 + ky) * WP + o
                            nc.tensor.matmul(
                                psvv,
                                vwt[:, pi * 128:(pi + 1) * 128].rearrange(
                                    "p (two m) -> p two m", two=2),
                                pair_rhs(pt, off, 0),
                                start=(pi == 0), stop=(pi == 5),
                                perf_mode=mybir.MatmulPerfMode.DoubleRow)
                            pi += 1
                    col = slice(band * 512, (band + 1) * 512)
                    if band % 2 == 0:
                        nc.vector.tensor_scalar(stq[:, col], psq[:], 1.0 / WSC, None,
                                                op0=mybir.AluOpType.mult)
                        nc.scalar.activation(stv[:, col], psv[:],
                                             mybir.ActivationFunctionType.Copy,
                                             scale=1.0 / WSC)
                    else:
                        nc.scalar.activation(stq[:, col], psq[:],
                                             mybir.ActivationFunctionType.Copy,
                                             scale=1.0 / WSC)
                        nc.vector.tensor_scalar(stv[:, col], psv[:], 1.0 / WSC, None,
                                                op0=mybir.AluOpType.mult)
                nc.sync.dma_start(qk.ap()[pr], stq[:])
                nc.sync.dma_start(vo.ap()[pr], stv[:])
    nc.compile()
    return nc


def _build_l2():
    """Attention, head-sharded (2 heads/core).

    Logits: 256 accumulating K=128 matmuls per head over host-packed d-major
    fp8 qT/kT tiles. Softmax on device. att@v as 64 N=512 matmuls with a
    block-diagonal [128,128] lhsT covering both heads at once."""
    nc = bacc.Bacc("TRN2", target_bir_lowering=False, debug=False)
    qt = nc.dram_tensor("qt", (2, 128, 16384), FP8, kind="ExternalInput")
    kt = nc.dram_tensor("kt", (2, 128, 16384), FP8, kind="ExternalInput")
    vst = nc.dram_tensor("vst", (128, HW * 2), BF16, kind="ExternalInput")
    mask = nc.dram_tensor("mask", (T, T), F32, kind="ExternalInput")
    ident = nc.dram_tensor("ident", (T, T), BF16, kind="ExternalInput")
    ys = nc.dram_tensor("ys", (128, HW * 2), BF16, kind="ExternalOutput")

    with tile.TileContext(nc) as tc:
        with tc.tile_pool(name="cst", bufs=1) as cst, \
             tc.tile_pool(name="qk", bufs=1) as qkpool, \
             tc.tile_pool(name="sm", bufs=2) as smpool, \
             tc.tile_pool(name="v", bufs=3) as vpool, \
             tc.tile_pool(name="yst", bufs=3) as ypool, \
             tc.tile_pool(name="pst", bufs=2, space="PSUM") as pstpool, \
             tc.tile_pool(name="psy", bufs=4, space="PSUM") as psypool, \
             tc.tile_pool(name="psl", bufs=1, space="PSUM") as pslpool:
            mask_t = cst.tile([T, T], F32, tag="mask")
            nc.sync.dma_start(mask_t[:], mask.ap())
            id_t = cst.tile([T, T], BF16, tag="ident")
            nc.sync.dma_start(id_t[:], ident.ap())

            qtl = qkpool.tile([128, 2 * 16384], FP8, tag="qtl")
            ktl = qkpool.tile([128, 2 * 16384], FP8, tag="ktl")
            for h in range(2):
                nc.scalar.dma_start(qtl[:, h * 16384:(h + 1) * 16384], qt.ap()[h])
                nc.scalar.dma_start(ktl[:, h * 16384:(h + 1) * 16384], kt.ap()[h])

            ld = qkpool.tile([128, 128], BF16, tag="ld")
            nc.vector.memset(ld[:], 0)

            for h in range(2):
                lg_ps = pslpool.tile([T, T], F32, name=f"lg{h}")
                for ck in range(256):
                    o = h * 16384 + ck * 64
                    nc.tensor.matmul(lg_ps[:], qtl[:, o:o + 64], ktl[:, o:o + 64],
                                     start=(ck == 0), stop=(ck == 255))
                lg = smpool.tile([T, T], F32, tag="lg")
                nc.vector.tensor_scalar(lg[:], lg_ps[:], SCALE, None,
                                        op0=mybir.AluOpType.mult)
                nc.vector.tensor_add(lg[:], lg[:], mask_t[:])
                mx = smpool.tile([T, 1], F32, tag="mx")
                nc.vector.reduce_max(mx[:], lg[:], axis=mybir.AxisListType.X, negate=True)
                e = smpool.tile([T, T], F32, tag="e")
                sm_acc = smpool.tile([T, 1], F32, tag="smacc")
                nc.scalar.activation(e[:], lg[:], mybir.ActivationFunctionType.Exp,
                                     bias=mx[:], scale=1.0, accum_out=sm_acc[:])
                rc = smpool.tile([T, 1], F32, tag="rc")
                nc.vector.reciprocal(rc[:], sm_acc[:])
                att = smpool.tile([T, T], BF16, tag="att")
                nc.vector.tensor_scalar(att[:], e[:], rc[:], None,
                                        op0=mybir.AluOpType.mult)
                ps_t = pstpool.tile([T, T], BF16, tag="pst")
                nc.tensor.transpose(ps_t[:], att[:], id_t[:])
                nc.vector.tensor_copy(ld[h * 64:h * 64 + 64, h * 64:h * 64 + 64], ps_t[:])

            for blk in range(4):
                vt = vpool.tile([128, 8192], BF16, tag="vt")
                nc.scalar.dma_start(vt[:], vst.ap()[:, blk * 8192:(blk + 1) * 8192])
                yst = ypool.tile([128, 8192], BF16, tag="yst")
                for j in range(16):
                    ps_y = psypool.tile([128, 512], F32, tag="psy")
                    nc.tensor.matmul(ps_y[:], ld[:], vt[:, j * 512:(j + 1) * 512],
                                     start=True, stop=True)
                    if j % 2 == 0:
                        nc.vector.tensor_copy(yst[:, j * 512:(j + 1) * 512], ps_y[:])
                    else:
                        nc.scalar.activation(yst[:, j * 512:(j + 1) * 512], ps_y[:],
                                             mybir.ActivationFunctionType.Copy)
                nc.sync.dma_start(ys.ap()[:, blk * 8192:(blk + 1) * 8192], yst[:])
    nc.compile()
    return nc


def _build_l3():
    """o-conv, image-sharded, bf16: partitions (img4, d2, c16), M (img4, j2, och16)=128.

    6 accumulating passes per psum tile: ky in {0,1,2} x o in {0,2}; rhs is the
    plane tile at AP offset (h+ky)*WP + o with col-pair stride 2. Bias added on
    host afterwards."""
    nc = bacc.Bacc("TRN2", target_bir_lowering=False, debug=False)
    planes = nc.dram_tensor("planes", (4, 128, PL3), BF16, kind="ExternalInput")
    lhsT = nc.dram_tensor("lhsT", (128, 6 * 128), BF16, kind="ExternalInput")
    out = nc.dram_tensor("out", (4, 128, 8192), BF16, kind="ExternalOutput")

    with tile.TileContext(nc) as tc:
        with tc.tile_pool(name="w", bufs=1) as wpool, \
             tc.tile_pool(name="pl", bufs=2) as plpool, \
             tc.tile_pool(name="st", bufs=2) as stpool, \
             tc.tile_pool(name="ps", bufs=4, space="PSUM") as pspool:
            wt = wpool.tile([128, 6 * 128], BF16, tag="wt")
            nc.sync.dma_start(wt[:], lhsT.ap())


            for g in range(4):
                pt = plpool.tile([128, PL3], BF16)
                nc.scalar.dma_start(pt[:], planes.ap()[g])
                stage = stpool.tile([128, 8192], BF16)
                for band in range(16):
                    ps = pspool.tile([128, 512], F32)
                    psv = ps[:].rearrange("p (h w) -> p h w", w=64)
                    first = True
                    for ky in range(3):
                        for oi, o in enumerate((0, 2)):
                            off = (band * 8 + ky) * WP + o
                            rhs = pt[0:128, off:off + 8 * WP].rearrange(
                                "p (h w) -> p h w", w=WP)[:, :, 0:128:2]
                            pi = ky * 2 + oi
                            nc.tensor.matmul(psv, wt[:, pi * 128:(pi + 1) * 128],
                                             rhs, start=first, stop=(ky == 2 and oi == 1))
                            first = False
                    if band % 2 == 0:
                        nc.vector.tensor_copy(stage[:, band * 512:(band + 1) * 512], ps[:])
                    else:
                        nc.scalar.activation(stage[:, band * 512:(band + 1) * 512], ps[:],
                                             mybir.ActivationFunctionType.Copy)
                nc.sync.dma_start(out.ap()[g], stage[:])
    nc.compile()
    return nc


def _get(name):
    if name not in _BUILD_CACHE:
        _BUILD_CACHE[name] = {"l1": _build_l1, "l2": _build_l2, "l3": _build_l3}[name]()
    return _BUILD_CACHE[name]


# ---------------- host-side packing ----------------

def _fp8_dual(a):
    """a float32 -> (hi, lo) fp8 arrays with hi + lo ~= a."""
    hi = a.astype(NPFP8)
    lo = (a - hi.astype(np.float32)).astype(NPFP8)
    return hi, lo


def _pack_l1_planes(imgs_chw):
    """imgs_chw: [16, C, H, W] f32 -> [8, 128, PL1] fp8, parts (xd2,i2,d2,c16)."""
    xh, xl = _fp8_dual(imgs_chw)
    flat = np.zeros((2, 16, C, HP + 1, WP), NPFP8)
    flat[0, :, :, 1:H + 1, 1:W + 1] = xh
    flat[1, :, :, 1:H + 1, 1:W + 1] = xl
    flat = flat.reshape(2, 16, C, (HP + 1) * WP)  # [xd, img, c, 17030]
    p = np.zeros((8, 2, 2, 2, C, PL1), NPFP8)     # [pr, xd, i, d, c, :]
    fl = flat.reshape(2, 8, 2, C, -1).transpose(1, 0, 2, 3, 4)  # [pr, xd, i, c, :]
    n = fl.shape[-1]
    p[:, :, :, 0, :, :n] = fl
    p[:, :, :, 1, :, :n - 1] = fl[..., 1:]
    return p.reshape(8, 128, PL1)


def _build_l1_qkw(wq, wk):
    """-> [128, 4*256] fp8; pass pi=(grp,oi); rows (xd,i,d,c); cols (kyp,(i,j,qk,och))."""
    wqs = (wq * WSC).astype(NPFP8).astype(np.float32)
    wks = (wk * WSC).astype(NPFP8).astype(np.float32)
    l = np.zeros((4, 2, 2, C, 2, 2, 2, 2, O), np.float32)  # [pi, d,c?, ...] build per (i)
    # dims: [pi, d, c, kyp, j, qk, och] then expand (xd, i) with blockdiag over i
    m = np.zeros((4, 2, C, 2, 2, 2, O), np.float32)  # [pi, d, c, kyp, j, qk, och]
    for grp in range(2):
        for oi, o in enumerate((0, 2)):
            pi = grp * 2 + oi
            for kyp in range(2):
                ky = grp * 2 + kyp
                if ky > 2:
                    continue
                for d in range(2):
                    for j in range(2):
                        kx = o + d - j
                        if 0 <= kx <= 2:
                            m[pi, d, :, kyp, j, 0, :] = wqs[:, :, ky, kx].T
                            m[pi, d, :, kyp, j, 1, :] = wks[:, :, ky, kx].T
    out = np.zeros((4, 2, 2, 2, C, 2, 2, 2, 2, O), np.float32)
    # [pi, xd, i, d, c, kyp, i', j, qk, och]
    for xd in range(2):
        for i in range(2):
            out[:, xd, i, :, :, :, i] = m
    out = out.reshape(4, 128, 2, 128).transpose(1, 0, 2, 3).reshape(128, 4 * 256)
    return out.astype(NPFP8)


def _build_l1_vw(wv):
    """-> [128, 6*128] fp8; pass pi=(ky,oi); rows (xd,i,d,c); pair (w_hi,w_lo)."""
    w0 = (wv * WSC).astype(NPFP8)
    w1 = (wv * WSC - w0.astype(np.float32)).astype(NPFP8)
    wds = [w0.astype(np.float32), w1.astype(np.float32)]
    m = np.zeros((6, 2, C, 2, 2, O), np.float32)  # [pi, d, c, wd, j, och]
    for ky in range(3):
        for oi, o in enumerate((0, 2)):
            pi = ky * 2 + oi
            for d in range(2):
                for j in range(2):
                    kx = o + d - j
                    if 0 <= kx <= 2:
                        for wd in range(2):
                            m[pi, d, :, wd, j, :] = wds[wd][:, :, ky, kx].T
    out = np.zeros((6, 2, 2, 2, C, 2, 2, 2, O), np.float32)  # [pi, xd, i, d, c, wd, i', j, och]
    for xd in range(2):
        for i in range(2):
            out[:, xd, i, :, :, :, i, :, :] = m
    out = out.reshape(6, 128, 2, 64).transpose(1, 0, 2, 3).reshape(128, 6 * 128)
    return out.astype(NPFP8)


def _unpack_l1(qk_res, v_res):
    """qk_res [8,128,8192] fp8, v_res [8,64,8192] bf16 -> q,k,v [16,16,HW] f32."""
    s = qk_res.astype(np.float32).reshape(8, 2, 2, 2, 16, 16, 8, 64)
    # [pr, i, j, qk, och, band, r, n] -> [pr, i, och, band, r, n, j]
    s = s.transpose(0, 1, 4, 5, 6, 7, 2, 3)  # pr i och band r n j qk
    q = np.ascontiguousarray(s[..., 0]).reshape(16, 16, HW)
    k = np.ascontiguousarray(s[..., 1]).reshape(16, 16, HW)
    sv = v_res.astype(np.float32).reshape(8, 2, 2, 16, 16, 8, 64)
    sv = sv.transpose(0, 1, 3, 4, 5, 6, 2)  # pr i och band r n j
    v = np.ascontiguousarray(sv).reshape(16, 16, HW)
    return q, k, v


def _pack_l3_planes(yimg16):
    """yimg16: [16, 16, H, W] float32 -> [4, 128, HP*WP] bf16 (img4, d2, c16)."""
    ypad = np.zeros((16, C, HP, WP), np.float32)
    ypad[:, :, 1:H + 1, 1:W + 1] = yimg16
    flat = ypad.reshape(16, C, HP * WP).astype(NPBF16)
    p = np.zeros((4, 4, 2, C, PL3), NPBF16)
    p[:, :, 0, :, :HP * WP] = flat.reshape(4, 4, C, HP * WP)
    p[:, :, 1, :, :HP * WP - 1] = flat.reshape(4, 4, C, HP * WP)[..., 1:]
    return p.reshape(4, 128, PL3)


def _build_l3_lhsT(wo):
    """wo: [O, C, 3, 3] -> [6, 128, 128] bf16; row (i,d,c), col (i,j,och)."""
    m = np.zeros((3, 2, 2, C, 2, O), np.float32)  # [ky, o_i, d, c, j, och]
    for ky in range(3):
        for oi, o in enumerate((0, 2)):
            for d in range(2):
                for j in range(2):
                    kx = o + d - j
                    if 0 <= kx <= 2:
                        m[ky, oi, d, :, j, :] = wo[:, :, ky, kx].T
    l = np.zeros((3, 2, 4, 2, C, 4, 2, O), np.float32)  # [ky,oi, i,d,c, i',j,och]
    for i in range(4):
        l[:, :, i, :, :, i] = m
    return np.ascontiguousarray(
        l.reshape(6, 128, 128).transpose(1, 0, 2)).reshape(128, 6 * 128).astype(NPBF16)


def _unpack_l3(o):
    """[4, 128, 8192] bf16 -> [16, 16, H, W] float32."""
    s = o.reshape(4, 4, 2, 16, 16, 8, 64).astype(np.float32)  # g i j och band r n
    s = s.transpose(0, 1, 3, 4, 5, 6, 2)  # g i och band r n j
    return np.ascontiguousarray(s).reshape(16, 16, 128, 128)


# ---------------- top level ----------------

def kernel(x, wq, bq, wk, bk, wv, bv, wo, bo):
    x, wq, bq, wk, bk, wv, bv, wo, bo = (
        np.asarray(a, np.float32) for a in (x, wq, bq, wk, bk, wv, bv, wo, bo))
    ximg = x.reshape(IMGS, C, H, W)
    cores = list(range(NCORES))

    # ---- L1: q/k/v convs, image-sharded (fp8 DoubleRow)
    qkw = _build_l1_qkw(wq, wk)
    vww = _build_l1_vw(wv)
    in_maps = [{"planes": _pack_l1_planes(ximg[c * IPC:(c + 1) * IPC]),
                "qkw": qkw, "vw": vww} for c in cores]
    res1 = bass_utils.run_bass_kernel_spmd(_get("l1"), in_maps, core_ids=cores)

    # assemble channel-major [B, 16, T, HW] f32 with biases
    q_all = np.empty((B, 16, T, HW), np.float32)
    k_all = np.empty_like(q_all)
    v_all = np.empty_like(q_all)
    for c in cores:
        q, k, v = _unpack_l1(res1.results[c]["qk_out"], res1.results[c]["v_out"])
        b0 = (c * IPC) // T
        t0 = (c * IPC) % T
        q_all[b0, :, t0:t0 + IPC] = q.transpose(1, 0, 2)
        k_all[b0, :, t0:t0 + IPC] = k.transpose(1, 0, 2)
        v_all[b0, :, t0:t0 + IPC] = v.transpose(1, 0, 2)
    q_all += bq[None, :, None, None]
    k_all += bk[None, :, None, None]
    v_all += bv[None, :, None, None]

    # ---- L2: attention, head-sharded (2 heads = 4 channels per core)
    mask = np.triu(np.full((T, T), -30000.0, np.float32), 1)
    ident = np.eye(T, dtype=NPBF16)
    in_maps = []
    for c in cores:
        b, g = c // 4, c % 4
        q4 = q_all[b, 4 * g:4 * g + 4].astype(np.float32)   # [4, T, HW]
        k4 = k_all[b, 4 * g:4 * g + 4].astype(np.float32)
        v4 = v_all[b, 4 * g:4 * g + 4].astype(np.float32)

        def dmaj(a):
            # [4,T,HW] -> [2 heads, d=2*HW, T] -> packed [2, 128, 16384] fp8
            aT = a.reshape(2, 2, T, HW).transpose(0, 1, 3, 2).reshape(2, 2 * HW, T)
            return np.ascontiguousarray(
                aT.reshape(2, 256, 128, T).transpose(0, 2, 1, 3).reshape(2, 128, 256 * T)
            ).astype(NPFP8)

        vstk = np.ascontiguousarray(
            v4.reshape(2, 2, T, HW).transpose(0, 2, 1, 3).reshape(128, 2 * HW)
        ).astype(NPBF16)
        in_maps.append({"qt": dmaj(q4), "kt": dmaj(k4), "vst": vstk,
                        "mask": mask, "ident": ident})
    res2 = bass_utils.run_bass_kernel_spmd(_get("l2"), in_maps, core_ids=cores)

    y_all = np.empty((B, 16, T, HW), NPBF16)
    for c in cores:
        b, g = c // 4, c % 4
        ys = res2.results[c]["ys"].reshape(2, T, 2, HW).transpose(0, 2, 1, 3)
        y_all[b, 4 * g:4 * g + 4] = ys.reshape(4, T, HW)
        # ---- L3: output conv, image-sharded
    yimg = y_all.astype(np.float32).transpose(0, 2, 1, 3).reshape(IMGS, 16, H, W)
    l3w = _build_l3_lhsT(wo)
    in_maps = [{"planes": _pack_l3_planes(yimg[c * IPC:(c + 1) * IPC]),
                "lhsT": l3w} for c in cores]
    res3 = bass_utils.run_bass_kernel_spmd(_get("l3"), in_maps, core_ids=cores)

    out = np.concatenate([_unpack_l3(res3.results[c]["out"]) for c in cores])
    out = out + bo.reshape(1, 16, 1, 1)
    return np.ascontiguousarray(out.reshape(B, T, O, H, W))


# revision 11
# speedup vs baseline: 2.1554x; 1.0047x over previous
"""Trainium2 Bass kernel for nn_CNNT_enhanced_denoising_runtime_53704271069472.

Computes, distributed across 8 NeuronCores:
    q/k/v = conv3x3(x, w?, b?)          (image-sharded: B*T=128 imgs, 16/core)
    att   = causal-softmax(q @ k^T / sqrt(D)) per (batch, head)
    y     = att @ v                      (head-sharded: 16 (b,head) pairs, 2/core)
    out   = conv3x3(y, wo, bo)           (image-sharded)

Three SPMD launches with host-side resharding between them. Convs are done as
matmuls over K = (3 kx-taps x 16 ch [+ ones bias row]) x 2 ky-rows = 97(+48)
against kx-pre-shifted zero-padded image planes built on the host; the 3x3
kernel's third ky row is a second accumulating matmul with an AP row offset.
Compute dtype bf16 (fp32 PSUM accumulation).
"""
import sys
import numpy as np

sys.path.insert(0, "/opt/trn_rl_repo")

import ml_dtypes  # noqa: E402
import concourse.bacc as bacc  # noqa: E402
import concourse.tile as tile  # noqa: E402
import concourse.bass as bass  # noqa: E402
from concourse import mybir, bass_utils  # noqa: E402

BF16 = mybir.dt.bfloat16
F32 = mybir.dt.float32
FP8 = mybir.dt.float8e4
NPBF16 = ml_dtypes.bfloat16
NPFP8 = ml_dtypes.float8_e4m3

B, T, C, H, W, O = 2, 64, 16, 128, 128, 16
HP, WP = H + 2, W + 2
HW = H * W
NH, HC = 8, 2
D = HC * HW
SCALE = float(1.0 / np.sqrt(np.float32(D)))
NCORES = 8
IMGS = B * T
IPC = IMGS // NCORES  # images per core
NPL = 98  # plane rows: 48 (ky0 kx-taps) + ones + 48 (ky1) + pad
PL3 = HP * WP + 2  # l3 plane free size (2 slack for o=2 shift at last band)
PL1 = (HP + 1) * WP + 2  # l1 plane: extra zero row + slack for ky-pair overrun
WSC = 64.0  # fp8 weight scale 2**6, folded out at psum copy

_BUILD_CACHE = {}


# ---------------- device programs ----------------

def _build_l1():
    """qkv convs, image-sharded, fp8 DoubleRow.

    Per 2 images one tile [(xd2, i2, d2, c16)=128, PL1] fp8: xd = (x_hi|x_lo)
    dual-fp8 halves, d in {0,1} column-shifted copies, per-partition planes of
    the zero-padded image. q+k fused: DR pair=(ky,ky+1) via free-dim stride WP,
    4 passes (kygrp2 x o in {0,2}), M=128=(i2,j2,qk2,och16). v: DR pair =
    (w_hi,w_lo) via stride-0 rhs pair (weight-dual), 6 passes (ky3 x o2),
    M=64=(i2,j2,och16). Weights are pre-scaled by WSC; copies scale back.
    Biases are added on the host."""
    nc = bacc.Bacc("TRN2", target_bir_lowering=False, debug=False)
    planes = nc.dram_tensor("planes", (IPC // 2, 128, PL1), FP8, kind="ExternalInput")
    qkw = nc.dram_tensor("qkw", (128, 4 * 256), FP8, kind="ExternalInput")
    vw = nc.dram_tensor("vw", (128, 6 * 128), FP8, kind="ExternalInput")
    qk = nc.dram_tensor("qk_out", (IPC // 2, 128, 8192), FP8, kind="ExternalOutput")
    vo = nc.dram_tensor("v_out", (IPC // 2, 64, 8192), BF16, kind="ExternalOutput")

    with tile.TileContext(nc) as tc:
        with tc.tile_pool(name="w", bufs=1) as wpool, \
             tc.tile_pool(name="pl", bufs=2) as plpool, \
             tc.tile_pool(name="stq", bufs=2) as stqpool, \
             tc.tile_pool(name="stv", bufs=2) as stvpool, \
             tc.tile_pool(name="ps", bufs=3, space="PSUM") as pspool:
            qw = wpool.tile([128, 4 * 256], FP8, tag="qw")
            nc.sync.dma_start(qw[:], qkw.ap())
            vwt = wpool.tile([128, 6 * 128], FP8, tag="vw")
            nc.sync.dma_start(vwt[:], vw.ap())

            def pair_rhs(pt, off, stride):
                base = pt[0:128, off:off + 9 * WP].rearrange(
                    "p (h w) -> p h w", w=WP)[:, 0:8, 0:128:2]
                r = base.unsqueeze(1).broadcast_to((128, 2, 8, 64))
                if stride:
                    r.ap[1] = [stride, 2]
                return r

            for pr in range(IPC // 2):
                pt = plpool.tile([128, PL1], FP8)
                nc.scalar.dma_start(pt[:], planes.ap()[pr])
                stq = stqpool.tile([128, 8192], FP8, tag="stq")
                stv = stvpool.tile([64, 8192], BF16, tag="stv")
                for band in range(16):
                    psq = pspool.tile([128, 512], F32, tag="psq")
                    psqv = psq[:].rearrange("p (h w) -> p h w", w=64)
                    pi = 0
                    for grp in range(2):
                        for o in (0, 2):
                            off = (band * 8 + grp * 2) * WP + o
                            nc.tensor.matmul(
                                psqv,
                                qw[:, pi * 256:(pi + 1) * 256].rearrange(
                                    "p (two m) -> p two m", two=2),
                                pair_rhs(pt, off, WP),
                                start=(pi == 0), stop=(pi == 3),
                                perf_mode=mybir.MatmulPerfMode.DoubleRow)
                            pi += 1
                    psv = pspool.tile([64, 512], F32, tag="psv")
                    psvv = psv[:].rearrange("p (h w) -> p h w", w=64)
                    pi = 0
                    for ky in range(3):
                        for o in (0, 2):
                            off = (band * 8 + ky) * WP + o
                            nc.tensor.matmul(
                                psvv,
                                vwt[:, pi * 128:(pi + 1) * 128].rearrange(
                                    "p (two m) -> p two m", two=2),
                                pair_rhs(pt, off, 0),
                                start=(pi == 0), stop=(pi == 5),
                                perf_mode=mybir.MatmulPerfMode.DoubleRow)
                            pi += 1
                    col = slice(band * 512, (band + 1) * 512)
                    if band % 2 == 0:
                        nc.vector.tensor_scalar(stq[:, col], psq[:], 1.0 / WSC, None,
                                                op0=mybir.AluOpType.mult)
                        nc.scalar.activation(stv[:, col], psv[:],
                                             mybir.ActivationFunctionType.Copy,
                                             scale=1.0 / WSC)
                    else:
                        nc.scalar.activation(stq[:, col], psq[:],
                                             mybir.ActivationFunctionType.Copy,
                                             scale=1.0 / WSC)
                        nc.vector.tensor_scalar(stv[:, col], psv[:], 1.0 / WSC, None,
                                                op0=mybir.AluOpType.mult)
                nc.sync.dma_start(qk.ap()[pr], stq[:])
                nc.sync.dma_start(vo.ap()[pr], stv[:])
    nc.compile()
    return nc


def _build_l2():
    """Attention, head-sharded (2 heads/core).

    Logits: 256 accumulating K=128 matmuls per head over host-packed d-major
    fp8 qT/kT tiles. Softmax on device. att@v as 64 N=512 matmuls with a
    block-diagonal [128,128] lhsT covering both heads at once."""
    nc = bacc.Bacc("TRN2", target_bir_lowering=False, debug=False)
    qt = nc.dram_tensor("qt", (2, 128, 16384), FP8, kind="ExternalInput")
    kt = nc.dram_tensor("kt", (2, 128, 16384), FP8, kind="ExternalInput")
    vst = nc.dram_tensor("vst", (128, HW * 2), BF16, kind="ExternalInput")
    mask = nc.dram_tensor("mask", (T, T), F32, kind="ExternalInput")
    ident = nc.dram_tensor("ident", (T, T), BF16, kind="ExternalInput")
    ys = nc.dram_tensor("ys", (128, HW * 2), BF16, kind="ExternalOutput")

    with tile.TileContext(nc) as tc:
        with tc.tile_pool(name="cst", bufs=1) as cst, \
             tc.tile_pool(name="qk", bufs=1) as qkpool, \
             tc.tile_pool(name="sm", bufs=2) as smpool, \
             tc.tile_pool(name="v", bufs=3) as vpool, \
             tc.tile_pool(name="yst", bufs=3) as ypool, \
             tc.tile_pool(name="pst", bufs=2, space="PSUM") as pstpool, \
             tc.tile_pool(name="psy", bufs=4, space="PSUM") as psypool, \
             tc.tile_pool(name="psl", bufs=1, space="PSUM") as pslpool:
            mask_t = cst.tile([T, T], F32, tag="mask")
            nc.sync.dma_start(mask_t[:], mask.ap())
            id_t = cst.tile([T, T], BF16, tag="ident")
            nc.sync.dma_start(id_t[:], ident.ap())

            qtl = qkpool.tile([128, 2 * 16384], FP8, tag="qtl")
            ktl = qkpool.tile([128, 2 * 16384], FP8, tag="ktl")
            for h in range(2):
                nc.scalar.dma_start(qtl[:, h * 16384:(h + 1) * 16384], qt.ap()[h])
                nc.scalar.dma_start(ktl[:, h * 16384:(h + 1) * 16384], kt.ap()[h])

            ld = qkpool.tile([128, 128], BF16, tag="ld")
            nc.vector.memset(ld[:], 0)

            for h in range(2):
                lg_ps = pslpool.tile([T, T], F32, name=f"lg{h}")
                for ck in range(128):
                    o = h * 16384 + ck * 128
                    nc.tensor.matmul(
                        lg_ps[:],
                        qtl[:, o:o + 128].rearrange("p (two m) -> p two m", two=2),
                        ktl[:, o:o + 128].rearrange("p (two m) -> p two m", two=2),
                        start=(ck == 0), stop=(ck == 127),
                        perf_mode=mybir.MatmulPerfMode.DoubleRow)
                lg = smpool.tile([T, T], F32, tag="lg")
                nc.vector.tensor_scalar(lg[:], lg_ps[:], SCALE, None,
                                        op0=mybir.AluOpType.mult)
                nc.vector.tensor_add(lg[:], lg[:], mask_t[:])
                mx = smpool.tile([T, 1], F32, tag="mx")
                nc.vector.reduce_max(mx[:], lg[:], axis=mybir.AxisListType.X, negate=True)
                e = smpool.tile([T, T], F32, tag="e")
                sm_acc = smpool.tile([T, 1], F32, tag="smacc")
                nc.scalar.activation(e[:], lg[:], mybir.ActivationFunctionType.Exp,
                                     bias=mx[:], scale=1.0, accum_out=sm_acc[:])
                rc = smpool.tile([T, 1], F32, tag="rc")
                nc.vector.reciprocal(rc[:], sm_acc[:])
                att = smpool.tile([T, T], BF16, tag="att")
                nc.vector.tensor_scalar(att[:], e[:], rc[:], None,
                                        op0=mybir.AluOpType.mult)
                ps_t = pstpool.tile([T, T], BF16, tag="pst")
                nc.tensor.transpose(ps_t[:], att[:], id_t[:])
                nc.vector.tensor_copy(ld[h * 64:h * 64 + 64, h * 64:h * 64 + 64], ps_t[:])

            for blk in range(4):
                vt = vpool.tile([128, 8192], BF16, tag="vt")
                nc.scalar.dma_start(vt[:], vst.ap()[:, blk * 8192:(blk + 1) * 8192])
                yst = ypool.tile([128, 8192], BF16, tag="yst")
                for j in range(16):
                    ps_y = psypool.tile([128, 512], F32, tag="psy")
                    nc.tensor.matmul(ps_y[:], ld[:], vt[:, j * 512:(j + 1) * 512],
                                     start=True, stop=True)
                    if j % 2 == 0:
                        nc.vector.tensor_copy(yst[:, j * 512:(j + 1) * 512], ps_y[:])
                    else:
                        nc.scalar.activation(yst[:, j * 512:(j + 1) * 512], ps_y[:],
                                             mybir.ActivationFunctionType.Copy)
                nc.sync.dma_start(ys.ap()[:, blk * 8192:(blk + 1) * 8192], yst[:])
    nc.compile()
    return nc


def _build_l3():
    """o-conv, image-sharded, bf16: partitions (img4, d2, c16), M (img4, j2, och16)=128.

    6 accumulating passes per psum tile: ky in {0,1,2} x o in {0,2}; rhs is the
    plane tile at AP offset (h+ky)*WP + o with col-pair stride 2. Bias added on
    host afterwards."""
    nc = bacc.Bacc("TRN2", target_bir_lowering=False, debug=False)
    planes = nc.dram_tensor("planes", (4, 128, PL3), BF16, kind="ExternalInput")
    lhsT = nc.dram_tensor("lhsT", (128, 6 * 128), BF16, kind="ExternalInput")
    out = nc.dram_tensor("out", (4, 128, 8192), BF16, kind="ExternalOutput")

    with tile.TileContext(nc) as tc:
        with tc.tile_pool(name="w", bufs=1) as wpool, \
             tc.tile_pool(name="pl", bufs=2) as plpool, \
             tc.tile_pool(name="st", bufs=2) as stpool, \
             tc.tile_pool(name="ps", bufs=4, space="PSUM") as pspool:
            wt = wpool.tile([128, 6 * 128], BF16, tag="wt")
            nc.sync.dma_start(wt[:], lhsT.ap())


            for g in range(4):
                pt = plpool.tile([128, PL3], BF16)
                nc.scalar.dma_start(pt[:], planes.ap()[g])
                stage = stpool.tile([128, 8192], BF16)
                for band in range(16):
                    ps = pspool.tile([128, 512], F32)
                    psv = ps[:].rearrange("p (h w) -> p h w", w=64)
                    first = True
                    for ky in range(3):
                        for oi, o in enumerate((0, 2)):
                            off = (band * 8 + ky) * WP + o
                            rhs = pt[0:128, off:off + 8 * WP].rearrange(
                                "p (h w) -> p h w", w=WP)[:, :, 0:128:2]
                            pi = ky * 2 + oi
                            nc.tensor.matmul(psv, wt[:, pi * 128:(pi + 1) * 128],
                                             rhs, start=first, stop=(ky == 2 and oi == 1))
                            first = False
                    if band % 2 == 0:
                        nc.vector.tensor_copy(stage[:, band * 512:(band + 1) * 512], ps[:])
                    else:
                        nc.scalar.activation(stage[:, band * 512:(band + 1) * 512], ps[:],
                                             mybir.ActivationFunctionType.Copy)
                nc.sync.dma_start(out.ap()[g], stage[:])
    nc.compile()
    return nc


def _get(name):
    if name not in _BUILD_CACHE:
        _BUILD_CACHE[name] = {"l1": _build_l1, "l2": _build_l2, "l3": _build_l3}[name]()
    return _BUILD_CACHE[name]


# ---------------- host-side packing ----------------

def _fp8_dual(a):
    """a float32 -> (hi, lo) fp8 arrays with hi + lo ~= a."""
    hi = a.astype(NPFP8)
    lo = (a - hi.astype(np.float32)).astype(NPFP8)
    return hi, lo


def _pack_l1_planes(imgs_chw):
    """imgs_chw: [16, C, H, W] f32 -> [8, 128, PL1] fp8, parts (xd2,i2,d2,c16)."""
    xh, xl = _fp8_dual(imgs_chw)
    flat = np.zeros((2, 16, C, HP + 1, WP), NPFP8)
    flat[0, :, :, 1:H + 1, 1:W + 1] = xh
    flat[1, :, :, 1:H + 1, 1:W + 1] = xl
    flat = flat.reshape(2, 16, C, (HP + 1) * WP)  # [xd, img, c, 17030]
    p = np.zeros((8, 2, 2, 2, C, PL1), NPFP8)     # [pr, xd, i, d, c, :]
    fl = flat.reshape(2, 8, 2, C, -1).transpose(1, 0, 2, 3, 4)  # [pr, xd, i, c, :]
    n = fl.shape[-1]
    p[:, :, :, 0, :, :n] = fl
    p[:, :, :, 1, :, :n - 1] = fl[..., 1:]
    return p.reshape(8, 128, PL1)


def _build_l1_qkw(wq, wk):
    """-> [128, 4*256] fp8; pass pi=(grp,oi); rows (xd,i,d,c); cols (kyp,(i,j,qk,och))."""
    wqs = (wq * WSC).astype(NPFP8).astype(np.float32)
    wks = (wk * WSC).astype(NPFP8).astype(np.float32)
    l = np.zeros((4, 2, 2, C, 2, 2, 2, 2, O), np.float32)  # [pi, d,c?, ...] build per (i)
    # dims: [pi, d, c, kyp, j, qk, och] then expand (xd, i) with blockdiag over i
    m = np.zeros((4, 2, C, 2, 2, 2, O), np.float32)  # [pi, d, c, kyp, j, qk, och]
    for grp in range(2):
        for oi, o in enumerate((0, 2)):
            pi = grp * 2 + oi
            for kyp in range(2):
                ky = grp * 2 + kyp
                if ky > 2:
                    continue
                for d in range(2):
                    for j in range(2):
                        kx = o + d - j
                        if 0 <= kx <= 2:
                            m[pi, d, :, kyp, j, 0, :] = wqs[:, :, ky, kx].T
                            m[pi, d, :, kyp, j, 1, :] = wks[:, :, ky, kx].T
    out = np.zeros((4, 2, 2, 2, C, 2, 2, 2, 2, O), np.float32)
    # [pi, xd, i, d, c, kyp, i', j, qk, och]
    for xd in range(2):
        for i in range(2):
            out[:, xd, i, :, :, :, i] = m
    out = out.reshape(4, 128, 2, 128).transpose(1, 0, 2, 3).reshape(128, 4 * 256)
    return out.astype(NPFP8)


def _build_l1_vw(wv):
    """-> [128, 6*128] fp8; pass pi=(ky,oi); rows (xd,i,d,c); pair (w_hi,w_lo)."""
    w0 = (wv * WSC).astype(NPFP8)
    w1 = (wv * WSC - w0.astype(np.float32)).astype(NPFP8)
    wds = [w0.astype(np.float32), w1.astype(np.float32)]
    m = np.zeros((6, 2, C, 2, 2, O), np.float32)  # [pi, d, c, wd, j, och]
    for ky in range(3):
        for oi, o in enumerate((0, 2)):
            pi = ky * 2 + oi
            for d in range(2):
                for j in range(2):
                    kx = o + d - j
                    if 0 <= kx <= 2:
                        for wd in range(2):
                            m[pi, d, :, wd, j, :] = wds[wd][:, :, ky, kx].T
    out = np.zeros((6, 2, 2, 2, C, 2, 2, 2, O), np.float32)  # [pi, xd, i, d, c, wd, i', j, och]
    for xd in range(2):
        for i in range(2):
            out[:, xd, i, :, :, :, i, :, :] = m
    out = out.reshape(6, 128, 2, 64).transpose(1, 0, 2, 3).reshape(128, 6 * 128)
    return out.astype(NPFP8)


def _unpack_l1(qk_res, v_res):
    """qk_res [8,128,8192] fp8, v_res [8,64,8192] bf16 -> q,k,v [16,16,HW] f32."""
    s = qk_res.astype(np.float32).reshape(8, 2, 2, 2, 16, 16, 8, 64)
    # [pr, i, j, qk, och, band, r, n] -> [pr, i, och, band, r, n, j]
    s = s.transpose(0, 1, 4, 5, 6, 7, 2, 3)  # pr i och band r n j qk
    q = np.ascontiguousarray(s[..., 0]).reshape(16, 16, HW)
    k = np.ascontiguousarray(s[..., 1]).reshape(16, 16, HW)
    sv = v_res.astype(np.float32).reshape(8, 2, 2, 16, 16, 8, 64)
    sv = sv.transpose(0, 1, 3, 4, 5, 6, 2)  # pr i och band r n j
    v = np.ascontiguousarray(sv).reshape(16, 16, HW)
    return q, k, v


def _pack_l3_planes(yimg16):
    """yimg16: [16, 16, H, W] float32 -> [4, 128, HP*WP] bf16 (img4, d2, c16)."""
    ypad = np.zeros((16, C, HP, WP), np.float32)
    ypad[:, :, 1:H + 1, 1:W + 1] = yimg16
    flat = ypad.reshape(16, C, HP * WP).astype(NPBF16)
    p = np.zeros((4, 4, 2, C, PL3), NPBF16)
    p[:, :, 0, :, :HP * WP] = flat.reshape(4, 4, C, HP * WP)
    p[:, :, 1, :, :HP * WP - 1] = flat.reshape(4, 4, C, HP * WP)[..., 1:]
    return p.reshape(4, 128, PL3)


def _build_l3_lhsT(wo):
    """wo: [O, C, 3, 3] -> [6, 128, 128] bf16; row (i,d,c), col (i,j,och)."""
    m = np.zeros((3, 2, 2, C, 2, O), np.float32)  # [ky, o_i, d, c, j, och]
    for ky in range(3):
        for oi, o in enumerate((0, 2)):
            for d in range(2):
                for j in range(2):
                    kx = o + d - j
                    if 0 <= kx <= 2:
                        m[ky, oi, d, :, j, :] = wo[:, :, ky, kx].T
    l = np.zeros((3, 2, 4, 2, C, 4, 2, O), np.float32)  # [ky,oi, i,d,c, i',j,och]
    for i in range(4):
        l[:, :, i, :, :, i] = m
    return np.ascontiguousarray(
        l.reshape(6, 128, 128).transpose(1, 0, 2)).reshape(128, 6 * 128).astype(NPBF16)


def _unpack_l3(o):
    """[4, 128, 8192] bf16 -> [16, 16, H, W] float32."""
    s = o.reshape(4, 4, 2, 16, 16, 8, 64).astype(np.float32)  # g i j och band r n
    s = s.transpose(0, 1, 3, 4, 5, 6, 2)  # g i och band r n j
    return np.ascontiguousarray(s).reshape(16, 16, 128, 128)


# ---------------- top level ----------------

def kernel(x, wq, bq, wk, bk, wv, bv, wo, bo):
    x, wq, bq, wk, bk, wv, bv, wo, bo = (
        np.asarray(a, np.float32) for a in (x, wq, bq, wk, bk, wv, bv, wo, bo))
    ximg = x.reshape(IMGS, C, H, W)
    cores = list(range(NCORES))

    # ---- L1: q/k/v convs, image-sharded (fp8 DoubleRow)
    qkw = _build_l1_qkw(wq, wk)
    vww = _build_l1_vw(wv)
    in_maps = [{"planes": _pack_l1_planes(ximg[c * IPC:(c + 1) * IPC]),
                "qkw": qkw, "vw": vww} for c in cores]
    res1 = bass_utils.run_bass_kernel_spmd(_get("l1"), in_maps, core_ids=cores)

    # assemble channel-major [B, 16, T, HW] f32 with biases
    q_all = np.empty((B, 16, T, HW), np.float32)
    k_all = np.empty_like(q_all)
    v_all = np.empty_like(q_all)
    for c in cores:
        q, k, v = _unpack_l1(res1.results[c]["qk_out"], res1.results[c]["v_out"])
        b0 = (c * IPC) // T
        t0 = (c * IPC) % T
        q_all[b0, :, t0:t0 + IPC] = q.transpose(1, 0, 2)
        k_all[b0, :, t0:t0 + IPC] = k.transpose(1, 0, 2)
        v_all[b0, :, t0:t0 + IPC] = v.transpose(1, 0, 2)
    q_all += bq[None, :, None, None]
    k_all += bk[None, :, None, None]
    v_all += bv[None, :, None, None]

    # ---- L2: attention, head-sharded (2 heads = 4 channels per core)
    mask = np.triu(np.full((T, T), -30000.0, np.float32), 1)
    ident = np.eye(T, dtype=NPBF16)
    in_maps = []
    for c in cores:
        b, g = c // 4, c % 4
        q4 = q_all[b, 4 * g:4 * g + 4].astype(np.float32)   # [4, T, HW]
        k4 = k_all[b, 4 * g:4 * g + 4].astype(np.float32)
        v4 = v_all[b, 4 * g:4 * g + 4].astype(np.float32)

        def dmaj(a):
            # [4,T,HW] -> [2 heads, d=2*HW, T] -> packed [2, 128, 16384] fp8
            aT = a.reshape(2, 2, T, HW).transpose(0, 1, 3, 2).reshape(2, 2 * HW, T)
            return np.ascontiguousarray(
                aT.reshape(2, 256, 128, T).transpose(0, 2, 1, 3).reshape(2, 128, 256 * T)
            ).astype(NPFP8)

        vstk = np.ascontiguousarray(
            v4.reshape(2, 2, T, HW).transpose(0, 2, 1, 3).reshape(128, 2 * HW)
        ).astype(NPBF16)
        in_maps.append({"qt": dmaj(q4), "kt": dmaj(k4), "vst": vstk,
                        "mask": mask, "ident": ident})
    res2 = bass_utils.run_bass_kernel_spmd(_get("l2"), in_maps, core_ids=cores)

    y_all = np.empty((B, 16, T, HW), NPBF16)
    for c in cores:
        b, g = c // 4, c % 4
        ys = res2.results[c]["ys"].reshape(2, T, 2, HW).transpose(0, 2, 1, 3)
        y_all[b, 4 * g:4 * g + 4] = ys.reshape(4, T, HW)
        # ---- L3: output conv, image-sharded
    yimg = y_all.astype(np.float32).transpose(0, 2, 1, 3).reshape(IMGS, 16, H, W)
    l3w = _build_l3_lhsT(wo)
    in_maps = [{"planes": _pack_l3_planes(yimg[c * IPC:(c + 1) * IPC]),
                "lhsT": l3w} for c in cores]
    res3 = bass_utils.run_bass_kernel_spmd(_get("l3"), in_maps, core_ids=cores)

    out = np.concatenate([_unpack_l3(res3.results[c]["out"]) for c in cores])
    out = out + bo.reshape(1, 16, 1, 1)
    return np.ascontiguousarray(out.reshape(B, T, O, H, W))


# revision 13
# speedup vs baseline: 2.2959x; 1.0652x over previous
"""Trainium2 Bass kernel for nn_CNNT_enhanced_denoising_runtime_53704271069472.

Computes, distributed across 8 NeuronCores:
    q/k/v = conv3x3(x, w?, b?)          (image-sharded: B*T=128 imgs, 16/core)
    att   = causal-softmax(q @ k^T / sqrt(D)) per (batch, head)
    y     = att @ v                      (head-sharded: 16 (b,head) pairs, 2/core)
    out   = conv3x3(y, wo, bo)           (image-sharded)

Three SPMD launches with host-side resharding between them. Convs are done as
matmuls over K = (3 kx-taps x 16 ch [+ ones bias row]) x 2 ky-rows = 97(+48)
against kx-pre-shifted zero-padded image planes built on the host; the 3x3
kernel's third ky row is a second accumulating matmul with an AP row offset.
Compute dtype bf16 (fp32 PSUM accumulation).
"""
import sys
import numpy as np

sys.path.insert(0, "/opt/trn_rl_repo")

import ml_dtypes  # noqa: E402
import concourse.bacc as bacc  # noqa: E402
import concourse.tile as tile  # noqa: E402
import concourse.bass as bass  # noqa: E402
from concourse import mybir, bass_utils  # noqa: E402

BF16 = mybir.dt.bfloat16
F32 = mybir.dt.float32
FP8 = mybir.dt.float8e4
NPBF16 = ml_dtypes.bfloat16
NPFP8 = ml_dtypes.float8_e4m3

B, T, C, H, W, O = 2, 64, 16, 128, 128, 16
HP, WP = H + 2, W + 2
HW = H * W
NH, HC = 8, 2
D = HC * HW
SCALE = float(1.0 / np.sqrt(np.float32(D)))
NCORES = 8
IMGS = B * T
IPC = IMGS // NCORES  # images per core
NPL = 98  # plane rows: 48 (ky0 kx-taps) + ones + 48 (ky1) + pad
PL3 = HP * WP + 2  # l3 plane free size (2 slack for o=2 shift at last band)
PL1 = (HP + 1) * WP + 2  # l1 plane: extra zero row + slack for ky-pair overrun
WSC = 64.0  # fp8 weight scale 2**6, folded out at psum copy

_BUILD_CACHE = {}


# ---------------- device programs ----------------

def _build_l1():
    """qkv convs, image-sharded, fp8 DoubleRow.

    Per 2 images one tile [(xd2, i2, d2, c16)=128, PL1] fp8: xd = (x_hi|x_lo)
    dual-fp8 halves, d in {0,1} column-shifted copies, per-partition planes of
    the zero-padded image. q+k fused: DR pair=(ky,ky+1) via free-dim stride WP,
    4 passes (kygrp2 x o in {0,2}), M=128=(i2,j2,qk2,och16). v: DR pair =
    (w_hi,w_lo) via stride-0 rhs pair (weight-dual), 6 passes (ky3 x o2),
    M=64=(i2,j2,och16). Weights are pre-scaled by WSC; copies scale back.
    Biases are added on the host."""
    nc = bacc.Bacc("TRN2", target_bir_lowering=False, debug=False)
    planes = nc.dram_tensor("planes", (IPC // 2, 128, PL1), FP8, kind="ExternalInput")
    qkw = nc.dram_tensor("qkw", (128, 4 * 256), FP8, kind="ExternalInput")
    vw = nc.dram_tensor("vw", (128, 6 * 128), FP8, kind="ExternalInput")
    qk = nc.dram_tensor("qk_out", (IPC // 2, 128, 8192), FP8, kind="ExternalOutput")
    vo = nc.dram_tensor("v_out", (IPC // 2, 64, 8192), BF16, kind="ExternalOutput")

    with tile.TileContext(nc) as tc:
        with tc.tile_pool(name="w", bufs=1) as wpool, \
             tc.tile_pool(name="pl", bufs=2) as plpool, \
             tc.tile_pool(name="stq", bufs=2) as stqpool, \
             tc.tile_pool(name="stv", bufs=2) as stvpool, \
             tc.tile_pool(name="ps", bufs=4, space="PSUM") as pspool:
            qw = wpool.tile([128, 4 * 256], FP8, tag="qw")
            nc.sync.dma_start(qw[:], qkw.ap())
            vwt = wpool.tile([128, 6 * 128], FP8, tag="vw")
            nc.sync.dma_start(vwt[:], vw.ap())

            def pair_rhs(pt, off, stride):
                base = pt[0:128, off:off + 9 * WP].rearrange(
                    "p (h w) -> p h w", w=WP)[:, 0:8, 0:128:2]
                r = base.unsqueeze(1).broadcast_to((128, 2, 8, 64))
                if stride:
                    r.ap[1] = [stride, 2]
                return r

            CH1 = 4 * 8 * WP
            for pr in range(IPC // 2):
                pt = plpool.tile([128, PL1], FP8)
                for ck in range(4):
                    lo = ck * CH1
                    hi = min(lo + CH1 + 4 * WP + 2, PL1)
                    nc.scalar.dma_start(pt[:, lo:hi], planes.ap()[pr][:, lo:hi])
                stq = stqpool.tile([128, 8192], FP8, tag="stq")
                stv = stvpool.tile([64, 8192], BF16, tag="stv")
                for band in range(16):
                    psq = pspool.tile([128, 512], F32, tag="psq")
                    psqv = psq[:].rearrange("p (h w) -> p h w", w=64)
                    pi = 0
                    for grp in range(2):
                        for o in (0, 2):
                            off = (band * 8 + grp * 2) * WP + o
                            nc.tensor.matmul(
                                psqv,
                                qw[:, pi * 256:(pi + 1) * 256].rearrange(
                                    "p (two m) -> p two m", two=2),
                                pair_rhs(pt, off, WP),
                                start=(pi == 0), stop=(pi == 3),
                                perf_mode=mybir.MatmulPerfMode.DoubleRow)
                            pi += 1
                    psv = pspool.tile([64, 512], F32, tag="psv")
                    psvv = psv[:].rearrange("p (h w) -> p h w", w=64)
                    pi = 0
                    for ky in range(3):
                        for o in (0, 2):
                            off = (band * 8 + ky) * WP + o
                            nc.tensor.matmul(
                                psvv,
                                vwt[:, pi * 128:(pi + 1) * 128].rearrange(
                                    "p (two m) -> p two m", two=2),
                                pair_rhs(pt, off, 0),
                                start=(pi == 0), stop=(pi == 5),
                                perf_mode=mybir.MatmulPerfMode.DoubleRow)
                            pi += 1
                    col = slice(band * 512, (band + 1) * 512)
                    if band % 2 == 0:
                        nc.vector.tensor_scalar(stq[:, col], psq[:], 1.0 / WSC, None,
                                                op0=mybir.AluOpType.mult)
                        nc.scalar.activation(stv[:, col], psv[:],
                                             mybir.ActivationFunctionType.Copy,
                                             scale=1.0 / WSC)
                    else:
                        nc.scalar.activation(stq[:, col], psq[:],
                                             mybir.ActivationFunctionType.Copy,
                                             scale=1.0 / WSC)
                        nc.vector.tensor_scalar(stv[:, col], psv[:], 1.0 / WSC, None,
                                                op0=mybir.AluOpType.mult)
                nc.sync.dma_start(qk.ap()[pr], stq[:])
                nc.sync.dma_start(vo.ap()[pr], stv[:])
    nc.compile()
    return nc


def _build_l2():
    """Attention, head-sharded (2 heads/core).

    Logits: 256 accumulating K=128 matmuls per head over host-packed d-major
    fp8 qT/kT tiles. Softmax on device. att@v as 64 N=512 matmuls with a
    block-diagonal [128,128] lhsT covering both heads at once."""
    nc = bacc.Bacc("TRN2", target_bir_lowering=False, debug=False)
    qt = nc.dram_tensor("qt", (2, 128, 16384), FP8, kind="ExternalInput")
    kt = nc.dram_tensor("kt", (2, 128, 16384), FP8, kind="ExternalInput")
    vst = nc.dram_tensor("vst", (128, HW * 2), BF16, kind="ExternalInput")
    mask = nc.dram_tensor("mask", (T, T), F32, kind="ExternalInput")
    ident = nc.dram_tensor("ident", (T, T), BF16, kind="ExternalInput")
    ys = nc.dram_tensor("ys", (128, HW * 2), BF16, kind="ExternalOutput")

    with tile.TileContext(nc) as tc:
        with tc.tile_pool(name="cst", bufs=1) as cst, \
             tc.tile_pool(name="qk", bufs=1) as qkpool, \
             tc.tile_pool(name="sm", bufs=2) as smpool, \
             tc.tile_pool(name="v", bufs=3) as vpool, \
             tc.tile_pool(name="yst", bufs=3) as ypool, \
             tc.tile_pool(name="pst", bufs=2, space="PSUM") as pstpool, \
             tc.tile_pool(name="psy", bufs=4, space="PSUM") as psypool, \
             tc.tile_pool(name="psl", bufs=1, space="PSUM") as pslpool:
            mask_t = cst.tile([T, T], F32, tag="mask")
            nc.sync.dma_start(mask_t[:], mask.ap())
            id_t = cst.tile([T, T], BF16, tag="ident")
            nc.sync.dma_start(id_t[:], ident.ap())

            qtl = qkpool.tile([128, 2 * 16384], FP8, tag="qtl")
            ktl = qkpool.tile([128, 2 * 16384], FP8, tag="ktl")
            for h in range(2):
                for ck in range(4):
                    lo, hi = ck * 4096, (ck + 1) * 4096
                    nc.scalar.dma_start(qtl[:, h * 16384 + lo:h * 16384 + hi],
                                        qt.ap()[h][:, lo:hi])
                    nc.scalar.dma_start(ktl[:, h * 16384 + lo:h * 16384 + hi],
                                        kt.ap()[h][:, lo:hi])

            ld = qkpool.tile([128, 128], BF16, tag="ld")
            nc.vector.memset(ld[:], 0)

            for h in range(2):
                lg_ps = pslpool.tile([T, T], F32, name=f"lg{h}")
                for ck in range(128):
                    o = h * 16384 + ck * 128
                    nc.tensor.matmul(
                        lg_ps[:],
                        qtl[:, o:o + 128].rearrange("p (two m) -> p two m", two=2),
                        ktl[:, o:o + 128].rearrange("p (two m) -> p two m", two=2),
                        start=(ck == 0), stop=(ck == 127),
                        perf_mode=mybir.MatmulPerfMode.DoubleRow)
                lg = smpool.tile([T, T], F32, tag="lg")
                nc.vector.tensor_scalar(lg[:], lg_ps[:], SCALE, None,
                                        op0=mybir.AluOpType.mult)
                nc.vector.tensor_add(lg[:], lg[:], mask_t[:])
                mx = smpool.tile([T, 1], F32, tag="mx")
                nc.vector.reduce_max(mx[:], lg[:], axis=mybir.AxisListType.X, negate=True)
                e = smpool.tile([T, T], F32, tag="e")
                sm_acc = smpool.tile([T, 1], F32, tag="smacc")
                nc.scalar.activation(e[:], lg[:], mybir.ActivationFunctionType.Exp,
                                     bias=mx[:], scale=1.0, accum_out=sm_acc[:])
                rc = smpool.tile([T, 1], F32, tag="rc")
                nc.vector.reciprocal(rc[:], sm_acc[:])
                att = smpool.tile([T, T], BF16, tag="att")
                nc.vector.tensor_scalar(att[:], e[:], rc[:], None,
                                        op0=mybir.AluOpType.mult)
                ps_t = pstpool.tile([T, T], BF16, tag="pst")
                nc.tensor.transpose(ps_t[:], att[:], id_t[:])
                nc.vector.tensor_copy(ld[h * 64:h * 64 + 64, h * 64:h * 64 + 64], ps_t[:])

            for blk in range(4):
                vt = vpool.tile([128, 8192], BF16, tag="vt")
                nc.scalar.dma_start(vt[:], vst.ap()[:, blk * 8192:(blk + 1) * 8192])
                yst = ypool.tile([128, 8192], BF16, tag="yst")
                for j in range(16):
                    ps_y = psypool.tile([128, 512], F32, tag="psy")
                    nc.tensor.matmul(ps_y[:], ld[:], vt[:, j * 512:(j + 1) * 512],
                                     start=True, stop=True)
                    if j % 2 == 0:
                        nc.vector.tensor_copy(yst[:, j * 512:(j + 1) * 512], ps_y[:])
                    else:
                        nc.scalar.activation(yst[:, j * 512:(j + 1) * 512], ps_y[:],
                                             mybir.ActivationFunctionType.Copy)
                nc.sync.dma_start(ys.ap()[:, blk * 8192:(blk + 1) * 8192], yst[:])
    nc.compile()
    return nc


def _build_l3():
    """o-conv, image-sharded, bf16: partitions (img4, d2, c16), M (img4, j2, och16)=128.

    6 accumulating passes per psum tile: ky in {0,1,2} x o in {0,2}; rhs is the
    plane tile at AP offset (h+ky)*WP + o with col-pair stride 2. Bias added on
    host afterwards."""
    nc = bacc.Bacc("TRN2", target_bir_lowering=False, debug=False)
    planes = nc.dram_tensor("planes", (4, 64, PL3), BF16, kind="ExternalInput")
    lhsT = nc.dram_tensor("lhsT", (128, 6 * 128), BF16, kind="ExternalInput")
    out = nc.dram_tensor("out", (4, 128, 8192), BF16, kind="ExternalOutput")

    with tile.TileContext(nc) as tc:
        with tc.tile_pool(name="w", bufs=1) as wpool, \
             tc.tile_pool(name="pl", bufs=2) as plpool, \
             tc.tile_pool(name="st", bufs=2) as stpool, \
             tc.tile_pool(name="ps", bufs=8, space="PSUM") as pspool:
            wt = wpool.tile([128, 6 * 128], BF16, tag="wt")
            nc.sync.dma_start(wt[:], lhsT.ap())


            CH3 = 4 * 8 * WP  # 4 bands of columns per chunk
            for g in range(4):
                pt = plpool.tile([128, PL3], BF16)
                for ck in range(4):
                    lo = ck * CH3
                    hi = min(lo + CH3 + 3 * WP + 2, PL3)
                    nc.scalar.dma_start(pt[0:64, lo:hi], planes.ap()[g][:, lo:hi])
                    dhi = PL3 - 1 if ck == 3 else (ck + 1) * CH3
                    nc.vector.tensor_copy(pt[64:128, lo:dhi], pt[0:64, lo + 1:dhi + 1])
                stage = stpool.tile([128, 8192], BF16)
                for band in range(16):
                    ps = pspool.tile([128, 512], F32)
                    psv = ps[:].rearrange("p (h w) -> p h w", w=64)
                    first = True
                    for ky in range(3):
                        for oi, o in enumerate((0, 2)):
                            off = (band * 8 + ky) * WP + o
                            rhs = pt[0:128, off:off + 8 * WP].rearrange(
                                "p (h w) -> p h w", w=WP)[:, :, 0:128:2]
                            pi = ky * 2 + oi
                            nc.tensor.matmul(psv, wt[:, pi * 128:(pi + 1) * 128],
                                             rhs, start=first, stop=(ky == 2 and oi == 1))
                            first = False
                    if band % 2 == 0:
                        nc.vector.tensor_copy(stage[:, band * 512:(band + 1) * 512], ps[:])
                    else:
                        nc.scalar.activation(stage[:, band * 512:(band + 1) * 512], ps[:],
                                             mybir.ActivationFunctionType.Copy)
                nc.sync.dma_start(out.ap()[g], stage[:])
    nc.compile()
    return nc


def _get(name):
    if name not in _BUILD_CACHE:
        _BUILD_CACHE[name] = {"l1": _build_l1, "l2": _build_l2, "l3": _build_l3}[name]()
    return _BUILD_CACHE[name]


# ---------------- host-side packing ----------------

def _fp8_dual(a):
    """a float32 -> (hi, lo) fp8 arrays with hi + lo ~= a."""
    hi = a.astype(NPFP8)
    lo = (a - hi.astype(np.float32)).astype(NPFP8)
    return hi, lo


def _pack_l1_planes(imgs_chw):
    """imgs_chw: [16, C, H, W] f32 -> [8, 128, PL1] fp8, parts (xd2,i2,d2,c16)."""
    xh, xl = _fp8_dual(imgs_chw)
    flat = np.zeros((2, 16, C, HP + 1, WP), NPFP8)
    flat[0, :, :, 1:H + 1, 1:W + 1] = xh
    flat[1, :, :, 1:H + 1, 1:W + 1] = xl
    flat = flat.reshape(2, 16, C, (HP + 1) * WP)  # [xd, img, c, 17030]
    p = np.zeros((8, 2, 2, 2, C, PL1), NPFP8)     # [pr, xd, i, d, c, :]
    fl = flat.reshape(2, 8, 2, C, -1).transpose(1, 0, 2, 3, 4)  # [pr, xd, i, c, :]
    n = fl.shape[-1]
    p[:, :, :, 0, :, :n] = fl
    p[:, :, :, 1, :, :n - 1] = fl[..., 1:]
    return p.reshape(8, 128, PL1)


def _build_l1_qkw(wq, wk):
    """-> [128, 4*256] fp8; pass pi=(grp,oi); rows (xd,i,d,c); cols (kyp,(i,j,qk,och))."""
    wqs = (wq * WSC).astype(NPFP8).astype(np.float32)
    wks = (wk * WSC).astype(NPFP8).astype(np.float32)
    l = np.zeros((4, 2, 2, C, 2, 2, 2, 2, O), np.float32)  # [pi, d,c?, ...] build per (i)
    # dims: [pi, d, c, kyp, j, qk, och] then expand (xd, i) with blockdiag over i
    m = np.zeros((4, 2, C, 2, 2, 2, O), np.float32)  # [pi, d, c, kyp, j, qk, och]
    for grp in range(2):
        for oi, o in enumerate((0, 2)):
            pi = grp * 2 + oi
            for kyp in range(2):
                ky = grp * 2 + kyp
                if ky > 2:
                    continue
                for d in range(2):
                    for j in range(2):
                        kx = o + d - j
                        if 0 <= kx <= 2:
                            m[pi, d, :, kyp, j, 0, :] = wqs[:, :, ky, kx].T
                            m[pi, d, :, kyp, j, 1, :] = wks[:, :, ky, kx].T
    out = np.zeros((4, 2, 2, 2, C, 2, 2, 2, 2, O), np.float32)
    # [pi, xd, i, d, c, kyp, i', j, qk, och]
    for xd in range(2):
        for i in range(2):
            out[:, xd, i, :, :, :, i] = m
    out = out.reshape(4, 128, 2, 128).transpose(1, 0, 2, 3).reshape(128, 4 * 256)
    return out.astype(NPFP8)


def _build_l1_vw(wv):
    """-> [128, 6*128] fp8; pass pi=(ky,oi); rows (xd,i,d,c); pair (w_hi,w_lo)."""
    w0 = (wv * WSC).astype(NPFP8)
    w1 = (wv * WSC - w0.astype(np.float32)).astype(NPFP8)
    wds = [w0.astype(np.float32), w1.astype(np.float32)]
    m = np.zeros((6, 2, C, 2, 2, O), np.float32)  # [pi, d, c, wd, j, och]
    for ky in range(3):
        for oi, o in enumerate((0, 2)):
            pi = ky * 2 + oi
            for d in range(2):
                for j in range(2):
                    kx = o + d - j
                    if 0 <= kx <= 2:
                        for wd in range(2):
                            m[pi, d, :, wd, j, :] = wds[wd][:, :, ky, kx].T
    out = np.zeros((6, 2, 2, 2, C, 2, 2, 2, O), np.float32)  # [pi, xd, i, d, c, wd, i', j, och]
    for xd in range(2):
        for i in range(2):
            out[:, xd, i, :, :, :, i, :, :] = m
    out = out.reshape(6, 128, 2, 64).transpose(1, 0, 2, 3).reshape(128, 6 * 128)
    return out.astype(NPFP8)


def _unpack_l1(qk_res, v_res):
    """qk_res [8,128,8192] fp8, v_res [8,64,8192] bf16 -> q,k,v [16,16,HW] f32."""
    s = qk_res.astype(np.float32).reshape(8, 2, 2, 2, 16, 16, 8, 64)
    # [pr, i, j, qk, och, band, r, n] -> [pr, i, och, band, r, n, j]
    s = s.transpose(0, 1, 4, 5, 6, 7, 2, 3)  # pr i och band r n j qk
    q = np.ascontiguousarray(s[..., 0]).reshape(16, 16, HW)
    k = np.ascontiguousarray(s[..., 1]).reshape(16, 16, HW)
    sv = v_res.astype(np.float32).reshape(8, 2, 2, 16, 16, 8, 64)
    sv = sv.transpose(0, 1, 3, 4, 5, 6, 2)  # pr i och band r n j
    v = np.ascontiguousarray(sv).reshape(16, 16, HW)
    return q, k, v


def _pack_l3_planes(yimg16):
    """yimg16: [16, 16, H, W] float32 -> [4, 128, HP*WP] bf16 (img4, d2, c16)."""
    ypad = np.zeros((16, C, HP, WP), np.float32)
    ypad[:, :, 1:H + 1, 1:W + 1] = yimg16
    flat = ypad.reshape(16, C, HP * WP).astype(NPBF16)
    p = np.zeros((4, 4, C, PL3), NPBF16)
    p[:, :, :, :HP * WP] = flat.reshape(4, 4, C, HP * WP)
    return p.reshape(4, 64, PL3)


def _build_l3_lhsT(wo):
    """wo: [O, C, 3, 3] -> [6, 128, 128] bf16; row (i,d,c), col (i,j,och)."""
    m = np.zeros((3, 2, 2, C, 2, O), np.float32)  # [ky, o_i, d, c, j, och]
    for ky in range(3):
        for oi, o in enumerate((0, 2)):
            for d in range(2):
                for j in range(2):
                    kx = o + d - j
                    if 0 <= kx <= 2:
                        m[ky, oi, d, :, j, :] = wo[:, :, ky, kx].T
    l = np.zeros((3, 2, 2, 4, C, 4, 2, O), np.float32)  # [ky,oi, d,i,c, i',j,och]
    for i in range(4):
        l[:, :, :, i, :, i] = m.transpose(0, 1, 2, 3, 4, 5)[:, :, :, :, :, :]  # [ky,oi,d,c,j,och]
    return np.ascontiguousarray(
        l.reshape(6, 128, 128).transpose(1, 0, 2)).reshape(128, 6 * 128).astype(NPBF16)


def _unpack_l3(o):
    """[4, 128, 8192] bf16 -> [16, 16, H, W] float32."""
    s = o.reshape(4, 4, 2, 16, 16, 8, 64).astype(np.float32)  # g i j och band r n
    s = s.transpose(0, 1, 3, 4, 5, 6, 2)  # g i och band r n j
    return np.ascontiguousarray(s).reshape(16, 16, 128, 128)


# ---------------- top level ----------------

def kernel(x, wq, bq, wk, bk, wv, bv, wo, bo):
    x, wq, bq, wk, bk, wv, bv, wo, bo = (
        np.asarray(a, np.float32) for a in (x, wq, bq, wk, bk, wv, bv, wo, bo))
    ximg = x.reshape(IMGS, C, H, W)
    cores = list(range(NCORES))

    # ---- L1: q/k/v convs, image-sharded (fp8 DoubleRow)
    qkw = _build_l1_qkw(wq, wk)
    vww = _build_l1_vw(wv)
    in_maps = [{"planes": _pack_l1_planes(ximg[c * IPC:(c + 1) * IPC]),
                "qkw": qkw, "vw": vww} for c in cores]
    res1 = bass_utils.run_bass_kernel_spmd(_get("l1"), in_maps, core_ids=cores)

    # assemble channel-major [B, 16, T, HW] f32 with biases
    q_all = np.empty((B, 16, T, HW), np.float32)
    k_all = np.empty_like(q_all)
    v_all = np.empty_like(q_all)
    for c in cores:
        q, k, v = _unpack_l1(res1.results[c]["qk_out"], res1.results[c]["v_out"])
        b0 = (c * IPC) // T
        t0 = (c * IPC) % T
        q_all[b0, :, t0:t0 + IPC] = q.transpose(1, 0, 2)
        k_all[b0, :, t0:t0 + IPC] = k.transpose(1, 0, 2)
        v_all[b0, :, t0:t0 + IPC] = v.transpose(1, 0, 2)
    q_all += bq[None, :, None, None]
    k_all += bk[None, :, None, None]
    v_all += bv[None, :, None, None]

    # ---- L2: attention, head-sharded (2 heads = 4 channels per core)
    mask = np.triu(np.full((T, T), -30000.0, np.float32), 1)
    ident = np.eye(T, dtype=NPBF16)
    in_maps = []
    for c in cores:
        b, g = c // 4, c % 4
        q4 = q_all[b, 4 * g:4 * g + 4].astype(np.float32)   # [4, T, HW]
        k4 = k_all[b, 4 * g:4 * g + 4].astype(np.float32)
        v4 = v_all[b, 4 * g:4 * g + 4].astype(np.float32)

        def dmaj(a):
            # [4,T,HW] -> [2 heads, d=2*HW, T] -> packed [2, 128, 16384] fp8
            aT = a.reshape(2, 2, T, HW).transpose(0, 1, 3, 2).reshape(2, 2 * HW, T)
            return np.ascontiguousarray(
                aT.reshape(2, 256, 128, T).transpose(0, 2, 1, 3).reshape(2, 128, 256 * T)
            ).astype(NPFP8)

        vstk = np.ascontiguousarray(
            v4.reshape(2, 2, T, HW).transpose(0, 2, 1, 3).reshape(128, 2 * HW)
        ).astype(NPBF16)
        in_maps.append({"qt": dmaj(q4), "kt": dmaj(k4), "vst": vstk,
                        "mask": mask, "ident": ident})
    res2 = bass_utils.run_bass_kernel_spmd(_get("l2"), in_maps, core_ids=cores)

    y_all = np.empty((B, 16, T, HW), NPBF16)
    for c in cores:
        b, g = c // 4, c % 4
        ys = res2.results[c]["ys"].reshape(2, T, 2, HW).transpose(0, 2, 1, 3)
        y_all[b, 4 * g:4 * g + 4] = ys.reshape(4, T, HW)
        # ---- L3: output conv, image-sharded
    yimg = y_all.astype(np.float32).transpose(0, 2, 1, 3).reshape(IMGS, 16, H, W)
    l3w = _build_l3_lhsT(wo)
    in_maps = [{"planes": _pack_l3_planes(yimg[c * IPC:(c + 1) * IPC]),
                "lhsT": l3w} for c in cores]
    res3 = bass_utils.run_bass_kernel_spmd(_get("l3"), in_maps, core_ids=cores)

    out = np.concatenate([_unpack_l3(res3.results[c]["out"]) for c in cores])
    out = out + bo.reshape(1, 16, 1, 1)
    return np.ascontiguousarray(out.reshape(B, T, O, H, W))


# revision 16
# speedup vs baseline: 2.4067x; 1.0482x over previous
"""Trainium2 Bass kernel for nn_CNNT_enhanced_denoising_runtime_53704271069472.

Computes, distributed across 8 NeuronCores:
    q/k/v = conv3x3(x, w?, b?)          (image-sharded: B*T=128 imgs, 16/core)
    att   = causal-softmax(q @ k^T / sqrt(D)) per (batch, head)
    y     = att @ v                      (head-sharded: 16 (b,head) pairs, 2/core)
    out   = conv3x3(y, wo, bo)           (image-sharded)

Three SPMD launches with host-side resharding between them. Convs are done as
matmuls over K = (3 kx-taps x 16 ch [+ ones bias row]) x 2 ky-rows = 97(+48)
against kx-pre-shifted zero-padded image planes built on the host; the 3x3
kernel's third ky row is a second accumulating matmul with an AP row offset.
Compute dtype bf16 (fp32 PSUM accumulation).
"""
import sys
import numpy as np

sys.path.insert(0, "/opt/trn_rl_repo")

import ml_dtypes  # noqa: E402
import concourse.bacc as bacc  # noqa: E402
import concourse.tile as tile  # noqa: E402
import concourse.bass as bass  # noqa: E402
from concourse import mybir, bass_utils  # noqa: E402

BF16 = mybir.dt.bfloat16
F32 = mybir.dt.float32
FP8 = mybir.dt.float8e4
NPBF16 = ml_dtypes.bfloat16
NPFP8 = ml_dtypes.float8_e4m3

B, T, C, H, W, O = 2, 64, 16, 128, 128, 16
HP, WP = H + 2, W + 2
HW = H * W
NH, HC = 8, 2
D = HC * HW
SCALE = float(1.0 / np.sqrt(np.float32(D)))
NCORES = 8
IMGS = B * T
IPC = IMGS // NCORES  # images per core
NPL = 98  # plane rows: 48 (ky0 kx-taps) + ones + 48 (ky1) + pad
PL3 = HP * WP + 2  # l3 plane free size (2 slack for o=2 shift at last band)
PL1 = (HP + 1) * WP + 2  # l1 plane: extra zero row + slack for ky-pair overrun
WSC = 64.0  # fp8 weight scale 2**6, folded out at psum copy

_BUILD_CACHE = {}


# ---------------- device programs ----------------

def _build_l1():
    """qkv convs, image-sharded, fp8 DoubleRow.

    Per 2 images one tile [(xd2, i2, d2, c16)=128, PL1] fp8: xd = (x_hi|x_lo)
    dual-fp8 halves, d in {0,1} column-shifted copies, per-partition planes of
    the zero-padded image. q+k fused: DR pair=(ky,ky+1) via free-dim stride WP,
    4 passes (kygrp2 x o in {0,2}), M=128=(i2,j2,qk2,och16). v: DR pair =
    (w_hi,w_lo) via stride-0 rhs pair (weight-dual), 6 passes (ky3 x o2),
    M=64=(i2,j2,och16). Weights are pre-scaled by WSC; copies scale back.
    Biases are added on the host."""
    nc = bacc.Bacc("TRN2", target_bir_lowering=False, debug=False)
    planes = nc.dram_tensor("planes", (IPC // 2, 128, PL1), FP8, kind="ExternalInput")
    qkw = nc.dram_tensor("qkw", (128, 4 * 256), FP8, kind="ExternalInput")
    vw = nc.dram_tensor("vw", (128, 6 * 128), FP8, kind="ExternalInput")
    qk = nc.dram_tensor("qk_out", (IPC // 2, 128, 8192), FP8, kind="ExternalOutput")
    vo = nc.dram_tensor("v_out", (IPC // 2, 64, 8192), BF16, kind="ExternalOutput")

    with tile.TileContext(nc) as tc:
        with tc.tile_pool(name="w", bufs=1) as wpool, \
             tc.tile_pool(name="pl", bufs=2) as plpool, \
             tc.tile_pool(name="stq", bufs=3) as stqpool, \
             tc.tile_pool(name="stv", bufs=3) as stvpool, \
             tc.tile_pool(name="ps", bufs=4, space="PSUM") as pspool:
            qw = wpool.tile([128, 4 * 256], FP8, tag="qw")
            nc.sync.dma_start(qw[:], qkw.ap())
            vwt = wpool.tile([128, 6 * 128], FP8, tag="vw")
            nc.sync.dma_start(vwt[:], vw.ap())

            def pair_rhs(pt, off, stride):
                base = pt[0:128, off:off + 9 * WP].rearrange(
                    "p (h w) -> p h w", w=WP)[:, 0:8, 0:128:2]
                r = base.unsqueeze(1).broadcast_to((128, 2, 8, 64))
                if stride:
                    r.ap[1] = [stride, 2]
                return r

            CH1 = 4 * 8 * WP
            for pr in range(IPC // 2):
                pt = plpool.tile([128, PL1], FP8)
                for ck in range(4):
                    lo = ck * CH1
                    hi = min(lo + CH1 + 4 * WP + 2, PL1)
                    nc.scalar.dma_start(pt[:, lo:hi], planes.ap()[pr][:, lo:hi])
                stq = stqpool.tile([128, 8192], FP8, tag="stq")
                stv = stvpool.tile([64, 8192], BF16, tag="stv")
                for band in range(16):
                    psq = pspool.tile([128, 512], F32, tag="psq")
                    psqv = psq[:].rearrange("p (h w) -> p h w", w=64)
                    pi = 0
                    for grp in range(2):
                        for o in (0, 2):
                            off = (band * 8 + grp * 2) * WP + o
                            nc.tensor.matmul(
                                psqv,
                                qw[:, pi * 256:(pi + 1) * 256].rearrange(
                                    "p (two m) -> p two m", two=2),
                                pair_rhs(pt, off, WP),
                                start=(pi == 0), stop=(pi == 3),
                                perf_mode=mybir.MatmulPerfMode.DoubleRow)
                            pi += 1
                    psv = pspool.tile([64, 512], F32, tag="psv")
                    psvv = psv[:].rearrange("p (h w) -> p h w", w=64)
                    pi = 0
                    for ky in range(3):
                        for o in (0, 2):
                            off = (band * 8 + ky) * WP + o
                            nc.tensor.matmul(
                                psvv,
                                vwt[:, pi * 128:(pi + 1) * 128].rearrange(
                                    "p (two m) -> p two m", two=2),
                                pair_rhs(pt, off, 0),
                                start=(pi == 0), stop=(pi == 5),
                                perf_mode=mybir.MatmulPerfMode.DoubleRow)
                            pi += 1
                    col = slice(band * 512, (band + 1) * 512)
                    if band % 2 == 0:
                        nc.vector.tensor_scalar(stq[:, col], psq[:], 1.0 / WSC, None,
                                                op0=mybir.AluOpType.mult)
                        nc.scalar.activation(stv[:, col], psv[:],
                                             mybir.ActivationFunctionType.Copy,
                                             scale=1.0 / WSC)
                    else:
                        nc.scalar.activation(stq[:, col], psq[:],
                                             mybir.ActivationFunctionType.Copy,
                                             scale=1.0 / WSC)
                        nc.vector.tensor_scalar(stv[:, col], psv[:], 1.0 / WSC, None,
                                                op0=mybir.AluOpType.mult)
                for hh in range(4):
                    cs = slice(hh * 2048, (hh + 1) * 2048)
                    nc.sync.dma_start(qk.ap()[pr][:, cs], stq[:, cs])
                    nc.sync.dma_start(vo.ap()[pr][:, cs], stv[:, cs])
    nc.compile()
    return nc


def _build_l2():
    """Attention, head-sharded (2 heads/core).

    Logits: 256 accumulating K=128 matmuls per head over host-packed d-major
    fp8 qT/kT tiles. Softmax on device. att@v as 64 N=512 matmuls with a
    block-diagonal [128,128] lhsT covering both heads at once."""
    nc = bacc.Bacc("TRN2", target_bir_lowering=False, debug=False)
    qt = nc.dram_tensor("qt", (2, 128, 16384), FP8, kind="ExternalInput")
    kt = nc.dram_tensor("kt", (2, 128, 16384), FP8, kind="ExternalInput")
    vst = nc.dram_tensor("vst", (128, HW * 2), BF16, kind="ExternalInput")
    mask = nc.dram_tensor("mask", (T, T), F32, kind="ExternalInput")
    ident = nc.dram_tensor("ident", (T, T), BF16, kind="ExternalInput")
    ys = nc.dram_tensor("ys", (128, HW * 2), BF16, kind="ExternalOutput")

    with tile.TileContext(nc) as tc:
        with tc.tile_pool(name="cst", bufs=1) as cst, \
             tc.tile_pool(name="qk", bufs=1) as qkpool, \
             tc.tile_pool(name="sm", bufs=2) as smpool, \
             tc.tile_pool(name="v", bufs=4) as vpool, \
             tc.tile_pool(name="yst", bufs=4) as ypool, \
             tc.tile_pool(name="pst", bufs=2, space="PSUM") as pstpool, \
             tc.tile_pool(name="psy", bufs=4, space="PSUM") as psypool, \
             tc.tile_pool(name="psl", bufs=1, space="PSUM") as pslpool:
            mask_t = cst.tile([T, T], F32, tag="mask")
            nc.sync.dma_start(mask_t[:], mask.ap())
            id_t = cst.tile([T, T], BF16, tag="ident")
            nc.sync.dma_start(id_t[:], ident.ap())

            qtl = qkpool.tile([128, 2 * 16384], FP8, tag="qtl")
            ktl = qkpool.tile([128, 2 * 16384], FP8, tag="ktl")
            for h in range(2):
                for ck in range(4):
                    lo, hi = ck * 4096, (ck + 1) * 4096
                    nc.scalar.dma_start(qtl[:, h * 16384 + lo:h * 16384 + hi],
                                        qt.ap()[h][:, lo:hi])
                    nc.scalar.dma_start(ktl[:, h * 16384 + lo:h * 16384 + hi],
                                        kt.ap()[h][:, lo:hi])

            ld = qkpool.tile([128, 128], BF16, tag="ld")
            nc.vector.memset(ld[:], 0)

            for h in range(2):
                lg_ps = pslpool.tile([T, T], F32, name=f"lg{h}")
                for ck in range(128):
                    o = h * 16384 + ck * 128
                    nc.tensor.matmul(
                        lg_ps[:],
                        qtl[:, o:o + 128].rearrange("p (two m) -> p two m", two=2),
                        ktl[:, o:o + 128].rearrange("p (two m) -> p two m", two=2),
                        start=(ck == 0), stop=(ck == 127),
                        perf_mode=mybir.MatmulPerfMode.DoubleRow)
                lg = smpool.tile([T, T], F32, tag="lg")
                nc.vector.tensor_scalar(lg[:], lg_ps[:], SCALE, None,
                                        op0=mybir.AluOpType.mult)
                nc.vector.tensor_add(lg[:], lg[:], mask_t[:])
                mx = smpool.tile([T, 1], F32, tag="mx")
                nc.vector.reduce_max(mx[:], lg[:], axis=mybir.AxisListType.X, negate=True)
                e = smpool.tile([T, T], F32, tag="e")
                sm_acc = smpool.tile([T, 1], F32, tag="smacc")
                nc.scalar.activation(e[:], lg[:], mybir.ActivationFunctionType.Exp,
                                     bias=mx[:], scale=1.0, accum_out=sm_acc[:])
                rc = smpool.tile([T, 1], F32, tag="rc")
                nc.vector.reciprocal(rc[:], sm_acc[:])
                att = smpool.tile([T, T], BF16, tag="att")
                nc.vector.tensor_scalar(att[:], e[:], rc[:], None,
                                        op0=mybir.AluOpType.mult)
                ps_t = pstpool.tile([T, T], BF16, tag="pst")
                nc.tensor.transpose(ps_t[:], att[:], id_t[:])
                nc.vector.tensor_copy(ld[h * 64:h * 64 + 64, h * 64:h * 64 + 64], ps_t[:])

            for blk in range(4):
                vt = vpool.tile([128, 8192], BF16, tag="vt")
                nc.scalar.dma_start(vt[:], vst.ap()[:, blk * 8192:(blk + 1) * 8192])
                yst = ypool.tile([128, 8192], BF16, tag="yst")
                for j in range(16):
                    ps_y = psypool.tile([128, 512], F32, tag="psy")
                    nc.tensor.matmul(ps_y[:], ld[:], vt[:, j * 512:(j + 1) * 512],
                                     start=True, stop=True)
                    if j % 2 == 0:
                        nc.vector.tensor_copy(yst[:, j * 512:(j + 1) * 512], ps_y[:])
                    else:
                        nc.scalar.activation(yst[:, j * 512:(j + 1) * 512], ps_y[:],
                                             mybir.ActivationFunctionType.Copy)
                for hh in range(4):
                    cs = slice(blk * 8192 + hh * 2048, blk * 8192 + (hh + 1) * 2048)
                    nc.sync.dma_start(ys.ap()[:, cs], yst[:, hh * 2048:(hh + 1) * 2048])
    nc.compile()
    return nc


def _build_l3():
    """o-conv, image-sharded, bf16: partitions (img4, d2, c16), M (img4, j2, och16)=128.

    6 accumulating passes per psum tile: ky in {0,1,2} x o in {0,2}; rhs is the
    plane tile at AP offset (h+ky)*WP + o with col-pair stride 2. Bias added on
    host afterwards."""
    nc = bacc.Bacc("TRN2", target_bir_lowering=False, debug=False)
    planes = nc.dram_tensor("planes", (4, 64, PL3), BF16, kind="ExternalInput")
    lhsT = nc.dram_tensor("lhsT", (128, 6 * 128), BF16, kind="ExternalInput")
    out = nc.dram_tensor("out", (4, 128, 8192), BF16, kind="ExternalOutput")

    with tile.TileContext(nc) as tc:
        with tc.tile_pool(name="w", bufs=1) as wpool, \
             tc.tile_pool(name="pl", bufs=3) as plpool, \
             tc.tile_pool(name="st", bufs=3) as stpool, \
             tc.tile_pool(name="ps", bufs=8, space="PSUM") as pspool:
            wt = wpool.tile([128, 6 * 128], BF16, tag="wt")
            nc.sync.dma_start(wt[:], lhsT.ap())


            CH3 = 4 * 8 * WP  # 4 bands of columns per chunk
            for g in range(4):
                pt = plpool.tile([128, PL3], BF16)
                for ck in range(4):
                    lo = ck * CH3
                    hi = min(lo + CH3 + 3 * WP + 2, PL3)
                    nc.scalar.dma_start(pt[0:64, lo:hi], planes.ap()[g][:, lo:hi])
                    dhi = PL3 - 1 if ck == 3 else (ck + 1) * CH3
                    nc.vector.tensor_copy(pt[64:128, lo:dhi], pt[0:64, lo + 1:dhi + 1])
                stage = stpool.tile([128, 8192], BF16)
                for band in range(16):
                    ps = pspool.tile([128, 512], F32)
                    psv = ps[:].rearrange("p (h w) -> p h w", w=64)
                    first = True
                    for ky in range(3):
                        for oi, o in enumerate((0, 2)):
                            off = (band * 8 + ky) * WP + o
                            rhs = pt[0:128, off:off + 8 * WP].rearrange(
                                "p (h w) -> p h w", w=WP)[:, :, 0:128:2]
                            pi = ky * 2 + oi
                            nc.tensor.matmul(psv, wt[:, pi * 128:(pi + 1) * 128],
                                             rhs, start=first, stop=(ky == 2 and oi == 1))
                            first = False
                    if band % 2 == 0:
                        nc.vector.tensor_copy(stage[:, band * 512:(band + 1) * 512], ps[:])
                    else:
                        nc.scalar.activation(stage[:, band * 512:(band + 1) * 512], ps[:],
                                             mybir.ActivationFunctionType.Copy)
                for hh in range(4):
                    cs = slice(hh * 2048, (hh + 1) * 2048)
                    nc.sync.dma_start(out.ap()[g][:, cs], stage[:, cs])
    nc.compile()
    return nc


def _get(name):
    if name not in _BUILD_CACHE:
        _BUILD_CACHE[name] = {"l1": _build_l1, "l2": _build_l2, "l3": _build_l3}[name]()
    return _BUILD_CACHE[name]


# ---------------- host-side packing ----------------

def _fp8_dual(a):
    """a float32 -> (hi, lo) fp8 arrays with hi + lo ~= a."""
    hi = a.astype(NPFP8)
    lo = (a - hi.astype(np.float32)).astype(NPFP8)
    return hi, lo


def _pack_l1_planes(imgs_chw):
    """imgs_chw: [16, C, H, W] f32 -> [8, 128, PL1] fp8, parts (xd2,i2,d2,c16)."""
    xh, xl = _fp8_dual(imgs_chw)
    flat = np.zeros((2, 16, C, HP + 1, WP), NPFP8)
    flat[0, :, :, 1:H + 1, 1:W + 1] = xh
    flat[1, :, :, 1:H + 1, 1:W + 1] = xl
    flat = flat.reshape(2, 16, C, (HP + 1) * WP)  # [xd, img, c, 17030]
    p = np.zeros((8, 2, 2, 2, C, PL1), NPFP8)     # [pr, xd, i, d, c, :]
    fl = flat.reshape(2, 8, 2, C, -1).transpose(1, 0, 2, 3, 4)  # [pr, xd, i, c, :]
    n = fl.shape[-1]
    p[:, :, :, 0, :, :n] = fl
    p[:, :, :, 1, :, :n - 1] = fl[..., 1:]
    return p.reshape(8, 128, PL1)


def _build_l1_qkw(wq, wk):
    """-> [128, 4*256] fp8; pass pi=(grp,oi); rows (xd,i,d,c); cols (kyp,(i,j,qk,och))."""
    wqs = (wq * WSC).astype(NPFP8).astype(np.float32)
    wks = (wk * WSC).astype(NPFP8).astype(np.float32)
    l = np.zeros((4, 2, 2, C, 2, 2, 2, 2, O), np.float32)  # [pi, d,c?, ...] build per (i)
    # dims: [pi, d, c, kyp, j, qk, och] then expand (xd, i) with blockdiag over i
    m = np.zeros((4, 2, C, 2, 2, 2, O), np.float32)  # [pi, d, c, kyp, j, qk, och]
    for grp in range(2):
        for oi, o in enumerate((0, 2)):
            pi = grp * 2 + oi
            for kyp in range(2):
                ky = grp * 2 + kyp
                if ky > 2:
                    continue
                for d in range(2):
                    for j in range(2):
                        kx = o + d - j
                        if 0 <= kx <= 2:
                            m[pi, d, :, kyp, j, 0, :] = wqs[:, :, ky, kx].T
                            m[pi, d, :, kyp, j, 1, :] = wks[:, :, ky, kx].T
    out = np.zeros((4, 2, 2, 2, C, 2, 2, 2, 2, O), np.float32)
    # [pi, xd, i, d, c, kyp, i', j, qk, och]
    for xd in range(2):
        for i in range(2):
            out[:, xd, i, :, :, :, i] = m
    out = out.reshape(4, 128, 2, 128).transpose(1, 0, 2, 3).reshape(128, 4 * 256)
    return out.astype(NPFP8)


def _build_l1_vw(wv):
    """-> [128, 6*128] fp8; pass pi=(ky,oi); rows (xd,i,d,c); pair (w_hi,w_lo)."""
    w0 = (wv * WSC).astype(NPFP8)
    w1 = (wv * WSC - w0.astype(np.float32)).astype(NPFP8)
    wds = [w0.astype(np.float32), w1.astype(np.float32)]
    m = np.zeros((6, 2, C, 2, 2, O), np.float32)  # [pi, d, c, wd, j, och]
    for ky in range(3):
        for oi, o in enumerate((0, 2)):
            pi = ky * 2 + oi
            for d in range(2):
                for j in range(2):
                    kx = o + d - j
                    if 0 <= kx <= 2:
                        for wd in range(2):
                            m[pi, d, :, wd, j, :] = wds[wd][:, :, ky, kx].T
    out = np.zeros((6, 2, 2, 2, C, 2, 2, 2, O), np.float32)  # [pi, xd, i, d, c, wd, i', j, och]
    for xd in range(2):
        for i in range(2):
            out[:, xd, i, :, :, :, i, :, :] = m
    out = out.reshape(6, 128, 2, 64).transpose(1, 0, 2, 3).reshape(128, 6 * 128)
    return out.astype(NPFP8)


def _unpack_l1(qk_res, v_res):
    """qk_res [8,128,8192] fp8, v_res [8,64,8192] bf16 -> q,k,v [16,16,HW] f32."""
    s = qk_res.astype(np.float32).reshape(8, 2, 2, 2, 16, 16, 8, 64)
    # [pr, i, j, qk, och, band, r, n] -> [pr, i, och, band, r, n, j]
    s = s.transpose(0, 1, 4, 5, 6, 7, 2, 3)  # pr i och band r n j qk
    q = np.ascontiguousarray(s[..., 0]).reshape(16, 16, HW)
    k = np.ascontiguousarray(s[..., 1]).reshape(16, 16, HW)
    sv = v_res.astype(np.float32).reshape(8, 2, 2, 16, 16, 8, 64)
    sv = sv.transpose(0, 1, 3, 4, 5, 6, 2)  # pr i och band r n j
    v = np.ascontiguousarray(sv).reshape(16, 16, HW)
    return q, k, v


def _pack_l3_planes(yimg16):
    """yimg16: [16, 16, H, W] float32 -> [4, 128, HP*WP] bf16 (img4, d2, c16)."""
    ypad = np.zeros((16, C, HP, WP), np.float32)
    ypad[:, :, 1:H + 1, 1:W + 1] = yimg16
    flat = ypad.reshape(16, C, HP * WP).astype(NPBF16)
    p = np.zeros((4, 4, C, PL3), NPBF16)
    p[:, :, :, :HP * WP] = flat.reshape(4, 4, C, HP * WP)
    return p.reshape(4, 64, PL3)


def _build_l3_lhsT(wo):
    """wo: [O, C, 3, 3] -> [6, 128, 128] bf16; row (i,d,c), col (i,j,och)."""
    m = np.zeros((3, 2, 2, C, 2, O), np.float32)  # [ky, o_i, d, c, j, och]
    for ky in range(3):
        for oi, o in enumerate((0, 2)):
            for d in range(2):
                for j in range(2):
                    kx = o + d - j
                    if 0 <= kx <= 2:
                        m[ky, oi, d, :, j, :] = wo[:, :, ky, kx].T
    l = np.zeros((3, 2, 2, 4, C, 4, 2, O), np.float32)  # [ky,oi, d,i,c, i',j,och]
    for i in range(4):
        l[:, :, :, i, :, i] = m.transpose(0, 1, 2, 3, 4, 5)[:, :, :, :, :, :]  # [ky,oi,d,c,j,och]
    return np.ascontiguousarray(
        l.reshape(6, 128, 128).transpose(1, 0, 2)).reshape(128, 6 * 128).astype(NPBF16)


def _unpack_l3(o):
    """[4, 128, 8192] bf16 -> [16, 16, H, W] float32."""
    s = o.reshape(4, 4, 2, 16, 16, 8, 64).astype(np.float32)  # g i j och band r n
    s = s.transpose(0, 1, 3, 4, 5, 6, 2)  # g i och band r n j
    return np.ascontiguousarray(s).reshape(16, 16, 128, 128)


# ---------------- top level ----------------

def kernel(x, wq, bq, wk, bk, wv, bv, wo, bo):
    x, wq, bq, wk, bk, wv, bv, wo, bo = (
        np.asarray(a, np.float32) for a in (x, wq, bq, wk, bk, wv, bv, wo, bo))
    ximg = x.reshape(IMGS, C, H, W)
    cores = list(range(NCORES))

    # ---- L1: q/k/v convs, image-sharded (fp8 DoubleRow)
    qkw = _build_l1_qkw(wq, wk)
    vww = _build_l1_vw(wv)
    in_maps = [{"planes": _pack_l1_planes(ximg[c * IPC:(c + 1) * IPC]),
                "qkw": qkw, "vw": vww} for c in cores]
    res1 = bass_utils.run_bass_kernel_spmd(_get("l1"), in_maps, core_ids=cores)

    # assemble channel-major [B, 16, T, HW] f32 with biases
    q_all = np.empty((B, 16, T, HW), np.float32)
    k_all = np.empty_like(q_all)
    v_all = np.empty_like(q_all)
    for c in cores:
        q, k, v = _unpack_l1(res1.results[c]["qk_out"], res1.results[c]["v_out"])
        b0 = (c * IPC) // T
        t0 = (c * IPC) % T
        q_all[b0, :, t0:t0 + IPC] = q.transpose(1, 0, 2)
        k_all[b0, :, t0:t0 + IPC] = k.transpose(1, 0, 2)
        v_all[b0, :, t0:t0 + IPC] = v.transpose(1, 0, 2)
    q_all += bq[None, :, None, None]
    k_all += bk[None, :, None, None]
    v_all += bv[None, :, None, None]

    # ---- L2: attention, head-sharded (2 heads = 4 channels per core)
    mask = np.triu(np.full((T, T), -30000.0, np.float32), 1)
    ident = np.eye(T, dtype=NPBF16)
    in_maps = []
    for c in cores:
        b, g = c // 4, c % 4
        q4 = q_all[b, 4 * g:4 * g + 4].astype(np.float32)   # [4, T, HW]
        k4 = k_all[b, 4 * g:4 * g + 4].astype(np.float32)
        v4 = v_all[b, 4 * g:4 * g + 4].astype(np.float32)

        def dmaj(a):
            # [4,T,HW] -> [2 heads, d=2*HW, T] -> packed [2, 128, 16384] fp8
            aT = a.reshape(2, 2, T, HW).transpose(0, 1, 3, 2).reshape(2, 2 * HW, T)
            return np.ascontiguousarray(
                aT.reshape(2, 256, 128, T).transpose(0, 2, 1, 3).reshape(2, 128, 256 * T)
            ).astype(NPFP8)

        vstk = np.ascontiguousarray(
            v4.reshape(2, 2, T, HW).transpose(0, 2, 1, 3).reshape(128, 2 * HW)
        ).astype(NPBF16)
        in_maps.append({"qt": dmaj(q4), "kt": dmaj(k4), "vst": vstk,
                        "mask": mask, "ident": ident})
    res2 = bass_utils.run_bass_kernel_spmd(_get("l2"), in_maps, core_ids=cores)

    y_all = np.empty((B, 16, T, HW), NPBF16)
    for c in cores:
        b, g = c // 4, c % 4
        ys = res2.results[c]["ys"].reshape(2, T, 2, HW).transpose(0, 2, 1, 3)
        y_all[b, 4 * g:4 * g + 4] = ys.reshape(4, T, HW)
        # ---- L3: output conv, image-sharded
    yimg = y_all.astype(np.float32).transpose(0, 2, 1, 3).reshape(IMGS, 16, H, W)
    l3w = _build_l3_lhsT(wo)
    in_maps = [{"planes": _pack_l3_planes(yimg[c * IPC:(c + 1) * IPC]),
                "lhsT": l3w} for c in cores]
    res3 = bass_utils.run_bass_kernel_spmd(_get("l3"), in_maps, core_ids=cores)

    out = np.concatenate([_unpack_l3(res3.results[c]["out"]) for c in cores])
    out = out + bo.reshape(1, 16, 1, 1)
    return np.ascontiguousarray(out.reshape(B, T, O, H, W))
